# revision 2
# baseline (speedup 1.0000x reference)
"""GCN (EnhancedTaintFlowGNN) on 8 Trainium2 NeuronCores.

Sharding: 32 graphs/core (batch sorted -> contiguous node ranges). Per GCN
layer: M = h @ W locally (PE), AllGather M, pull-side SpMM via dma_gather
from 8 owner-aligned int16 tables + PE segment-matmuls (norm folded into the
segment coefficients). Self-loops preloaded into PSUM by ACT scaled by
dinv^2. Layer 0 gathers rows of T0 = emb @ w0 (vocab table, replicated on
every core) so no exchange is needed. Pooling: masked DVE mean/max on
transposed features; classifier MLP on PE.
"""
import numpy as np
import ml_dtypes

import concourse.bass as bass
import concourse.bacc as bacc
import concourse.tile as tile
from concourse import mybir
from concourse.bass_utils import run_bass_kernel_spmd
from concourse.masks import make_identity

P = 128
N, E, G, V, D, H = 150000, 300000, 256, 1000, 128, 256
CORES = 8
GPC = G // CORES
GT = 16        # dst-tiles per psum group
MAXCALL = 8    # max 128-slot blocks per dma_gather call
BF = ml_dtypes.bfloat16
F32 = np.float32


def _idx16_layout(idx):
    n = idx.shape[0]
    s = n // 16
    out = np.zeros((P, s), np.int16)
    blk = idx.reshape(s, 16).T.astype(np.int16)
    for g in range(8):
        out[g * 16:(g + 1) * 16, :] = blk
    return out


def _build_stream(counts2d, ngrp, tiles):
    """counts2d [tiles, n_own]. Returns (segs, total_slots).
    segs: per (group, owner): (owner, slot_off, nblk, incidences)
    incidences: (local_blk, tile, a, b) row-ranges inside each block."""
    n_own = counts2d.shape[1]
    segs = []
    off = 0
    for g in range(ngrp):
        tl = list(range(g * GT, min((g + 1) * GT, tiles)))
        for o in range(n_own):
            cnts = [int(counts2d[t, o]) for t in tl]
            seglen = sum(cnts)
            nblk = max(1, (seglen + P - 1) // P)
            inc = []
            r = 0
            for t, k in zip(tl, cnts):
                if k == 0:
                    continue
                r0, r1 = r, r + k
                for b in range(r0 // P, (r1 - 1) // P + 1):
                    a = max(r0 - b * P, 0)
                    bb = min(r1 - b * P, P)
                    if bb > a:
                        inc.append((b, t, a, bb))
                r = r1
            segs.append((o, off, nblk, inc))
            off += nblk * P
    return segs, off


def _make_plan(segs, ngrp, inc_base):
    """Per group: calls [(owner, slot_off, nblk, blk0)] and tiles
    [(t, [(inc_id, bglob)])]; inc ids sequential in builder order."""
    per_g = len(segs) // ngrp
    plan = []
    inc_id = inc_base
    for g in range(ngrp):
        gsegs = segs[g * per_g:(g + 1) * per_g]
        calls = []
        for (o, off, nblk, inc) in gsegs:
            done = 0
            while done < nblk:
                nb = min(MAXCALL, nblk - done)
                calls.append((o, off + done * P, nb, off // P + done))
                done += nb
        tset = sorted({t for (_, _, _, inc) in gsegs for (_, t, _, _) in inc})
        tiles = []
        for t in tset:
            lst = []
            for (o, off, nblk, inc) in gsegs:
                for (b, tt, a, bb) in inc:
                    if tt == t:
                        lst.append((inc_id, off // P + b, a, bb))
                        inc_id += 1
            tiles.append((t, lst))
        plan.append((calls, tiles))
    return plan, inc_id


def _preprocess(node_ids, edge_index, batch):
    node_ids = np.asarray(node_ids).astype(np.int64)
    src_g = np.asarray(edge_index)[0].astype(np.int64)
    dst_g = np.asarray(edge_index)[1].astype(np.int64)
    batch = np.asarray(batch).astype(np.int64)

    deg = 1.0 + np.bincount(dst_g, minlength=N).astype(np.float64)
    dinv = (1.0 / np.sqrt(deg)).astype(F32)
    selfnorm = (dinv * dinv).astype(F32)

    gcnt = np.bincount(batch, minlength=G)
    goff = np.zeros(G + 1, np.int64)
    goff[1:] = np.cumsum(gcnt)
    node_start = np.array([int(goff[c * GPC]) for c in range(CORES + 1)])
    n_c = node_start[1:] - node_start[:-1]
    TILES = int(np.ceil(n_c.max() / P))
    SLAB = TILES * P
    assert SLAB <= 32767, SLAB
    NGRP = (TILES + GT - 1) // GT

    e_oc = np.searchsorted(node_start[1:], dst_g, side="right")
    e_os = np.searchsorted(node_start[1:], src_g, side="right")

    cores = []
    for c in range(CORES):
        m = e_oc == c
        cores.append(dict(src=src_g[m], dstl=dst_g[m] - node_start[c],
                          own=e_os[m], n=int(n_c[c]), nlo=int(node_start[c])))

    # uniform per-(tile, owner) counts
    K1 = np.zeros((TILES, CORES), np.int64)
    K0 = np.zeros((TILES, 1), np.int64)
    for c in range(CORES):
        t_of = cores[c]["dstl"] // P
        cnt = np.zeros((TILES, CORES), np.int64)
        np.add.at(cnt, (t_of, cores[c]["own"]), 1)
        np.maximum(K1, cnt, out=K1)
        c0 = np.bincount(t_of, minlength=TILES).reshape(-1, 1)
        np.maximum(K0, c0, out=K0)
    K0 = K0 + P  # layer-0 self-loop slots

    segs1, S1 = _build_stream(K1, NGRP, TILES)
    segs0, S0 = _build_stream(K0, NGRP, TILES)
    NB0, NB1 = S0 // P, S1 // P
    plan0, ninc0 = _make_plan(segs0, NGRP, 0)
    plan1, ninc1 = _make_plan(segs1, NGRP, ninc0)

    # graph cover ranges (uniform across cores)
    glo_all = np.stack([goff[c * GPC:(c + 1) * GPC] - node_start[c]
                        for c in range(CORES)])   # [CORES, GPC]
    ghi_all = np.stack([goff[c * GPC + 1:(c + 1) * GPC + 1] - node_start[c]
                        for c in range(CORES)])
    lo_fix = glo_all.min(0)
    hi_fix = ghi_all.max(0)
    MAXCOV = int((hi_fix - lo_fix).max())
    MAXCOV = (MAXCOV + 31) // 32 * 32
    cov_len = np.minimum(MAXCOV, SLAB - lo_fix)

    per_core = []
    for c in range(CORES):
        cd = cores[c]
        t_of = cd["dstl"] // P
        idx0 = np.zeros(S0, np.int16)
        idx1 = np.zeros(S1, np.int16)
        scol0 = np.zeros(S0, np.int64)
        scoef0 = np.zeros(S0, F32)
        scol1 = np.zeros(S1, np.int64)
        scoef1 = np.zeros(S1, F32)

        # --- L123 stream fill ---
        order = np.lexsort((cd["dstl"], t_of, cd["own"], t_of // GT))
        srcs = cd["src"][order]
        dstl = cd["dstl"][order]
        owns = cd["own"][order]
        tof = t_of[order]
        coefs = dinv[srcs] * dinv[dstl + cd["nlo"]]
        ptr = 0
        for (o, off, nblk, inc) in segs1:
            starts = {}
            for (b, t, a, bb) in inc:
                if t not in starts:
                    starts[t] = off + b * P + a
            for t in sorted(starts):
                p = starts[t]
                while ptr < len(srcs) and tof[ptr] == t and owns[ptr] == o:
                    idx1[p] = srcs[ptr] - node_start[o]
                    scol1[p] = dstl[ptr] % P
                    scoef1[p] = coefs[ptr]
                    p += 1
                    ptr += 1
        assert ptr == len(srcs), (c, ptr, len(srcs))

        # --- L0 stream fill (edges + self loops; table = T0 by vocab id) ---
        order0 = np.lexsort((cd["dstl"],))
        s0 = cd["src"][order0]
        d0 = cd["dstl"][order0]
        c0 = dinv[s0] * dinv[d0 + cd["nlo"]]
        ptr = 0
        for (o, off, nblk, inc) in segs0:
            starts = {}
            for (b, t, a, bb) in inc:
                if t not in starts:
                    starts[t] = off + b * P + a
            for t in sorted(starts):
                p = starts[t]
                while ptr < len(s0) and d0[ptr] // P == t:
                    idx0[p] = node_ids[s0[ptr]]
                    scol0[p] = d0[ptr] % P
                    scoef0[p] = c0[ptr]
                    p += 1
                    ptr += 1
                lo, hi = t * P, min((t + 1) * P, cd["n"])
                for dn in range(lo, hi):
                    gid = dn + cd["nlo"]
                    idx0[p] = node_ids[gid]
                    scol0[p] = dn % P
                    scoef0[p] = selfnorm[gid]
                    p += 1
        assert ptr == len(s0)

        seg = np.zeros((ninc1, P, P), F32)
        for plan, scol, scoef in ((plan0, scol0, scoef0),
                                  (plan1, scol1, scoef1)):
            for (calls, tiles) in plan:
                for (t, lst) in tiles:
                    for (inc_id, bglob, a, bb) in lst:
                        rows = np.arange(a, bb)
                        slots = bglob * P + rows
                        seg[inc_id, rows, scol[slots]] = scoef[slots]

        selfn = np.zeros((P, TILES), F32)
        ar = np.arange(cd["n"])
        selfn[ar % P, ar // P] = selfnorm[cd["nlo"]:cd["nlo"] + cd["n"]]

        # graph masks [GPC, P, MAXCOV] bf16 (replicated across partitions)
        mask = np.zeros((GPC, MAXCOV), F32)
        cinv = np.zeros(GPC, F32)
        for j in range(GPC):
            lo = int(glo_all[c, j]) - int(lo_fix[j])
            hi = int(ghi_all[c, j]) - int(lo_fix[j])
            mask[j, lo:hi] = 1.0
            cnt = int(ghi_all[c, j] - glo_all[c, j])
            cinv[j] = 1.0 / max(cnt, 1)
        maskb = np.broadcast_to(mask[:, None, :], (GPC, P, MAXCOV))

        per_core.append(dict(
            idx16=_idx16_layout(np.concatenate([idx0, idx1]).astype(np.int16)),
            seg=np.ascontiguousarray(seg.astype(BF)),
            selfn=selfn,
            mask=np.ascontiguousarray(maskb.astype(BF)),
            cinv=np.broadcast_to(cinv, (P, GPC)).copy(),
        ))

    meta = dict(TILES=TILES, SLAB=SLAB, NGRP=NGRP, NB0=NB0, NB1=NB1,
                S0=S0, S1=S1, segs0=segs0, segs1=segs1,
                plan0=plan0, plan1=plan1, NINC=ninc1,
                lo_fix=lo_fix, cov_len=cov_len, MAXCOV=MAXCOV)
    return per_core, meta


def _build(meta):
    TILES, SLAB, NGRP = meta["TILES"], meta["SLAB"], meta["NGRP"]
    NB0, NB1 = meta["NB0"], meta["NB1"]
    MAXCOV = meta["MAXCOV"]
    f32, bf16, i16 = mybir.dt.float32, mybir.dt.bfloat16, mybir.dt.int16
    AF = mybir.ActivationFunctionType
    nc = bacc.Bacc("TRN2", target_bir_lowering=False, debug=False,
                   num_devices=CORES)

    SCOLS = (meta["S0"] + meta["S1"]) // 16
    idx_in = nc.dram_tensor("idx16", [P, SCOLS], i16, kind="ExternalInput")
    seg_in = nc.dram_tensor("seg", [meta["NINC"], P, P], bf16,
                            kind="ExternalInput")
    selfn_in = nc.dram_tensor("selfn", [P, TILES], f32, kind="ExternalInput")
    mask_in = nc.dram_tensor("mask", [GPC, P, MAXCOV], bf16,
                             kind="ExternalInput")
    cinv_in = nc.dram_tensor("cinv", [P, GPC], f32, kind="ExternalInput")
    embT_in = nc.dram_tensor("embT", [P, 1024], bf16, kind="ExternalInput")
    w0_in = nc.dram_tensor("w0", [P, H], bf16, kind="ExternalInput")
    wl_in = nc.dram_tensor("wl", [P, 3, 2, H], bf16, kind="ExternalInput")
    b_in = nc.dram_tensor("bias", [P, 4, H], f32, kind="ExternalInput")
    cw0_in = nc.dram_tensor("cw0", [P, 4, H], bf16, kind="ExternalInput")
    cb0_in = nc.dram_tensor("cb0", [GPC, H], f32, kind="ExternalInput")
    cw1_in = nc.dram_tensor("cw1", [P, 2, 2], bf16, kind="ExternalInput")
    cb1_in = nc.dram_tensor("cb1", [GPC, 2], f32, kind="ExternalInput")
    out = nc.dram_tensor("out", [GPC, 2], f32, kind="ExternalOutput")

    T0_d = nc.dram_tensor("T0d", [1024, H], bf16)

    with tile.TileContext(nc) as tc:
        with (
            tc.tile_pool(name="const", bufs=1) as cpool,
            tc.tile_pool(name="gat", bufs=10) as gpool,
            tc.tile_pool(name="segp", bufs=4) as segp,
            tc.tile_pool(name="selfp", bufs=4) as selfp,
            tc.tile_pool(name="work", bufs=6) as work,
            tc.tile_pool(name="hmp", bufs=2) as hmp,
            tc.tile_pool(name="ps", bufs=4, space="PSUM") as ps,
            tc.tile_pool(name="pstr", bufs=2, space="PSUM") as pstr,
            tc.tile_pool(name="psmm", bufs=2, space="PSUM") as psmm,
            tc.tile_pool(name="dram", bufs=1, space="DRAM") as dpool,
        ):
            idx_t = cpool.tile([P, SCOLS], i16)
            nc.sync.dma_start(out=idx_t[:], in_=idx_in[:, :])
            selfn_t = cpool.tile([P, TILES], f32)
            nc.sync.dma_start(out=selfn_t[:], in_=selfn_in[:, :])
            cinv_t = cpool.tile([P, GPC], f32)
            nc.sync.dma_start(out=cinv_t[:], in_=cinv_in[:, :])
            embT_t = cpool.tile([P, 1024], bf16)
            nc.sync.dma_start(out=embT_t[:], in_=embT_in[:, :])
            w0_t = cpool.tile([P, H], bf16)
            nc.sync.dma_start(out=w0_t[:], in_=w0_in[:, :])
            wl_t = cpool.tile([P, 3, 2, H], bf16)
            nc.sync.dma_start(out=wl_t[:], in_=wl_in[:, :, :, :])
            bias_t = cpool.tile([P, 4, H], f32)
            nc.sync.dma_start(out=bias_t[:], in_=b_in[:, :, :])

            h_T = nc.alloc_sbuf_tensor("hT", [P, 2, SLAB], bf16)
            ident = cpool.tile([P, P], bf16)
            make_identity(nc, ident[:])

            # ---- T0 = emb @ w0 ----
            for vb in range(8):
                pt0 = psmm.tile([P, H], f32, tag="mm")
                nc.tensor.matmul(out=pt0[:], lhsT=embT_t[:, vb * P:(vb + 1) * P],
                                 rhs=w0_t[:], start=True, stop=True)
                t0s = work.tile([P, H], bf16)
                nc.scalar.copy(out=t0s[:], in_=pt0[:])
                nc.sync.dma_start(out=T0_d[vb * P:(vb + 1) * P, :], in_=t0s[:])

            MB = dpool.tile([SLAB, H], bf16)
            MF = dpool.tile([CORES * SLAB, H], bf16)

            def spmm(layer):
                plan = meta["plan0"] if layer == 0 else meta["plan1"]
                base_slot = 0 if layer == 0 else meta["S0"]
                for (calls, tiles) in plan:
                    blk_ref = {}
                    for (o, soff, nb, blk0) in calls:
                        gt_ = gpool.tile([P, MAXCALL, H], bf16)
                        slot0 = base_slot + soff
                        if layer == 0:
                            tab = T0_d[0:1024, :]
                        else:
                            tab = MF[o * SLAB:(o + 1) * SLAB, :]
                        nc.gpsimd.dma_gather(
                            out_ap=gt_[:, 0:nb, :], in_ap=tab,
                            idxs_ap=idx_t[:, slot0 // 16:
                                          (slot0 + nb * P) // 16],
                            num_idxs=nb * P, num_idxs_reg=nb * P,
                            elem_size=H, queue_num=0)
                        for b in range(nb):
                            blk_ref[blk0 + b] = (gt_, b)
                    for (t, lst) in tiles:
                        pt = ps.tile([P, H], f32)
                        if layer > 0:
                            ssrc = selfp.tile([P, H], bf16)
                            nc.sync.dma_start(
                                out=ssrc[:],
                                in_=MB[t * P:(t + 1) * P, :])
                            nc.scalar.activation(
                                out=pt[:], in_=ssrc[:], func=AF.Copy,
                                scale=selfn_t[:, t:t + 1])
                        for n_i, (inc_id, bglob, a, bb) in enumerate(lst):
                            gt_, loc = blk_ref[bglob]
                            st = segp.tile([P, P], bf16)
                            nc.sync.dma_start(out=st[:],
                                              in_=seg_in[inc_id, :, :])
                            nc.tensor.matmul(
                                out=pt[:], lhsT=st[:],
                                rhs=gt_[:, loc, :],
                                start=(layer == 0 and n_i == 0),
                                stop=(n_i == len(lst) - 1),
                                skip_group_check=(layer > 0))
                        tmp = work.tile([P, H], f32)
                        nc.vector.tensor_add(out=tmp[:], in0=pt[:],
                                             in1=bias_t[:, layer, :])
                        tmpb = work.tile([P, H], bf16)
                        nc.scalar.activation(out=tmpb[:], in_=tmp[:],
                                             func=AF.Relu)
                        for fh in range(2):
                            ptr_ = pstr.tile([P, P], bf16)
                            nc.tensor.transpose(
                                out=ptr_[:], in_=tmpb[:, fh * P:(fh + 1) * P],
                                identity=ident[:])
                            nc.vector.tensor_copy(
                                out=h_T[:, fh, t * P:(t + 1) * P], in_=ptr_[:])

            spmm(0)
            for layer in (1, 2, 3):
                # M = h @ W  -> MB, then AllGather -> MF
                for t in range(TILES):
                    pm = psmm.tile([P, H], f32, tag="mm")
                    for fh in range(2):
                        nc.tensor.matmul(
                            out=pm[:],
                            lhsT=h_T[:, fh, t * P:(t + 1) * P],
                            rhs=wl_t[:, layer - 1, fh, :],
                            start=(fh == 0), stop=(fh == 1))
                    mt = work.tile([P, H], bf16)
                    nc.scalar.copy(out=mt[:], in_=pm[:])
                    nc.sync.dma_start(out=MB[t * P:(t + 1) * P, :], in_=mt[:])
                nc.gpsimd.collective_compute(
                    "AllGather", mybir.AluOpType.bypass,
                    replica_groups=[list(range(CORES))],
                    ins=[MB[:].opt()], outs=[MF[:].opt()])
                spmm(layer)

            # ---- pooling (masked mean/max on h_T) ----
            pooled = []
            for fh in range(2):
                mean_t = cpool.tile([P, GPC], f32, tag=f"mean{fh}")
                max_t = cpool.tile([P, GPC], f32, tag=f"max{fh}")
                nc.vector.memset(mean_t[:], 0.0)
                nc.vector.memset(max_t[:], 0.0)
                pooled.append((mean_t, max_t))
            lo_fix, cov_len = meta["lo_fix"], meta["cov_len"]
            for j in range(GPC):
                mk = hmp.tile([P, MAXCOV], bf16, tag="mask")
                ln = int(cov_len[j])
                nc.sync.dma_start(out=mk[:, 0:ln], in_=mask_in[j, :, 0:ln])
                for fh in range(2):
                    hm = hmp.tile([P, MAXCOV], f32, tag="hm")
                    lo = int(lo_fix[j])
                    nc.vector.tensor_mul(out=hm[:, 0:ln],
                                         in0=h_T[:, fh, lo:lo + ln],
                                         in1=mk[:, 0:ln])
                    nc.vector.tensor_reduce(
                        out=pooled[fh][0][:, j:j + 1], in_=hm[:, 0:ln],
                        axis=mybir.AxisListType.X, op=mybir.AluOpType.add)
                    nc.vector.tensor_reduce(
                        out=pooled[fh][1][:, j:j + 1], in_=hm[:, 0:ln],
                        axis=mybir.AxisListType.X, op=mybir.AluOpType.max)
            # scale means by 1/cnt, cast to bf16 lhsT chunks
            chunks = []
            for fh in range(2):
                mean_t, max_t = pooled[fh]
                nc.vector.tensor_mul(out=mean_t[:], in0=mean_t[:],
                                     in1=cinv_t[:])
            for (kind, fh) in ((0, 0), (0, 1), (1, 0), (1, 1)):
                src = pooled[fh][kind]
                cb = work.tile([P, GPC], bf16, tag=f"ch{kind}{fh}")
                nc.vector.tensor_copy(out=cb[:], in_=src[:])
                chunks.append(cb)

            # ---- classifier MLP ----
            cw0_t = cpool.tile([P, 4, H], bf16)
            nc.sync.dma_start(out=cw0_t[:], in_=cw0_in[:, :, :])
            cb0_t = cpool.tile([GPC, H], f32)
            nc.sync.dma_start(out=cb0_t[:], in_=cb0_in[:, :])
            cw1_t = cpool.tile([P, 2, 2], bf16)
            nc.sync.dma_start(out=cw1_t[:], in_=cw1_in[:, :, :])
            cb1_t = cpool.tile([GPC, 2], f32)
            nc.sync.dma_start(out=cb1_t[:], in_=cb1_in[:, :])

            ph_full = psmm.tile([P, H], f32, tag="mm")
            ph = ph_full[0:GPC, :]
            for k in range(4):
                nc.tensor.matmul(out=ph[:], lhsT=chunks[k][:],
                                 rhs=cw0_t[:, k, :],
                                 start=(k == 0), stop=(k == 3))
            hc1 = work.tile([GPC, H], f32, tag="hc1")
            nc.vector.tensor_add(out=hc1[:], in0=ph[:], in1=cb0_t[:])
            hcb = work.tile([GPC, H], bf16, tag="hcb")
            nc.scalar.activation(out=hcb[:], in_=hc1[:], func=AF.Relu)
            hTt = []
            for k in range(2):
                ptr_ = pstr.tile([P, P], bf16)
                nc.tensor.transpose(out=ptr_[0:P, 0:GPC],
                                    in_=hcb[:, k * P:(k + 1) * P],
                                    identity=ident[0:GPC, 0:GPC])
                ht = work.tile([P, GPC], bf16, tag=f"hTt{k}")
                nc.vector.tensor_copy(out=ht[:], in_=ptr_[0:P, 0:GPC])
                hTt.append(ht)
            pl_full = psmm.tile([P, H], f32, tag="mm")
            pl = pl_full[0:GPC, 0:2]
            for k in range(2):
                nc.tensor.matmul(out=pl[:], lhsT=hTt[k][:],
                                 rhs=cw1_t[:, k, :],
                                 start=(k == 0), stop=(k == 1))
            lg = work.tile([GPC, 2], f32, tag="lg")
            nc.vector.tensor_add(out=lg[:], in0=pl[:], in1=cb1_t[:])
            nc.sync.dma_start(out=out[:, :], in_=lg[:])
    nc.finalize()
    return nc


def kernel(node_ids, edge_index, batch, emb, w0, b0, w1, b1, w2, b2, w3, b3,
           cw0, cb0, cw1, cb1):
    per_core, meta = _preprocess(node_ids, edge_index, batch)
    nc = _build(meta)

    embT = np.zeros((P, 1024), F32)
    embT[:, :V] = np.asarray(emb, F32).T
    wlk = np.transpose(np.stack([np.asarray(w, F32).reshape(2, P, H)
                    for w in (w1, w2, w3)]), (2, 0, 1, 3)).copy()
    biases = np.stack([np.broadcast_to(np.asarray(b, F32), (P, H))
                       for b in (b0, b1, b2, b3)], axis=1).copy()
    ins = []
    for c in range(CORES):
        pc = per_core[c]
        ins.append(dict(
            idx16=pc["idx16"], seg=pc["seg"], selfn=pc["selfn"],
            mask=pc["mask"], cinv=pc["cinv"],
            embT=embT.astype(BF), w0=np.asarray(w0, F32).astype(BF),
            wl=wlk.astype(BF), bias=biases.astype(F32),
            cw0=np.transpose(np.asarray(cw0, F32).reshape(4, P, H), (1, 0, 2)).astype(BF),
            cb0=np.broadcast_to(np.asarray(cb0, F32), (GPC, H)).copy(),
            cw1=np.transpose(np.asarray(cw1, F32).reshape(2, P, 2), (1, 0, 2)).astype(BF),
            cb1=np.broadcast_to(np.asarray(cb1, F32), (GPC, 2)).copy(),
        ))
    trace = False
    try:  # register NTFF hook so exec_time_ns is measurable (best effort)
        import sys, types
        import antenv
        if "antenv.axon_hooks" not in sys.modules:
            hooks = types.ModuleType("antenv.axon_hooks")
            hooks._h = None
            hooks.set_axon_ntff_profile_hook = lambda h: setattr(hooks, "_h", h)
            hooks.get_axon_ntff_profile_hook = lambda: hooks._h
            sys.modules["antenv.axon_hooks"] = hooks
            antenv.axon_hooks = hooks
            from trn_agent_boot.trn_boot import _ntff_profile_via_ctypes
            hk = _ntff_profile_via_ctypes("/opt/axon/libaxon_pjrt.so")
            if hk is not None:
                hooks.set_axon_ntff_profile_hook(hk)
                trace = True
        else:
            trace = True
    except Exception:
        trace = False
    res = run_bass_kernel_spmd(nc, ins, core_ids=list(range(CORES)),
                               trace=trace)
    logits = np.concatenate([res.results[c]["out"] for c in range(CORES)], 0)
    globals()["last_exec_ns"] = res.exec_time_ns
    globals()["last_res"] = res
    return logits.astype(np.float32)



# revision 9
# speedup vs baseline: 1.7490x; 1.7490x over previous
"""GCN (EnhancedTaintFlowGNN) on 8 Trainium2 NeuronCores.

Sharding: 32 graphs/core (batch sorted -> contiguous node ranges). Per GCN
layer: M = h @ W locally (PE), AllGather M, pull-side SpMM via dma_gather
from 8 owner-aligned int16 tables + PE segment-matmuls (norm folded into the
segment coefficients). Self-loops preloaded into PSUM by ACT scaled by
dinv^2. Layer 0 gathers rows of T0 = emb @ w0 (vocab table, replicated on
every core) so no exchange is needed. Pooling: masked DVE mean/max on
transposed features; classifier MLP on PE.
"""
import numpy as np
import ml_dtypes

import concourse.bass as bass
import concourse.bacc as bacc
import concourse.tile as tile
from concourse import mybir
from concourse.bass_utils import run_bass_kernel_spmd
from concourse.masks import make_identity

P = 128
N, E, G, V, D, H = 150000, 300000, 256, 1000, 128, 256
CORES = 8
GPC = G // CORES
GT = 16        # dst-tiles per psum group
MAXCALL = 8    # max 128-slot blocks per dma_gather call
BF = ml_dtypes.bfloat16
F32 = np.float32


def _idx16_layout(idx):
    n = idx.shape[0]
    s = n // 16
    out = np.zeros((P, s), np.int16)
    blk = idx.reshape(s, 16).T.astype(np.int16)
    for g in range(8):
        out[g * 16:(g + 1) * 16, :] = blk
    return out


def _build_stream(counts2d, ngrp, tiles):
    """counts2d [tiles, n_own]. Returns (segs, total_slots).
    segs: per (group, owner): (owner, slot_off, nblk, incidences)
    incidences: (local_blk, tile, a, b) row-ranges inside each block."""
    n_own = counts2d.shape[1]
    segs = []
    off = 0
    for g in range(ngrp):
        tl = list(range(g * GT, min((g + 1) * GT, tiles)))
        for o in range(n_own):
            cnts = [int(counts2d[t, o]) for t in tl]
            seglen = sum(cnts)
            nblk = max(1, (seglen + P - 1) // P)
            inc = []
            r = 0
            for t, k in zip(tl, cnts):
                if k == 0:
                    continue
                r0, r1 = r, r + k
                for b in range(r0 // P, (r1 - 1) // P + 1):
                    a = max(r0 - b * P, 0)
                    bb = min(r1 - b * P, P)
                    if bb > a:
                        inc.append((b, t, a, bb))
                r = r1
            segs.append((o, off, nblk, inc))
            off += nblk * P
    return segs, off


def _make_plan(segs, ngrp, inc_base):
    """Per group: calls [(owner, slot_off, nblk, blk0)] and tiles
    [(t, [(inc_id, bglob)])]; inc ids sequential in builder order."""
    per_g = len(segs) // ngrp
    plan = []
    inc_id = inc_base
    for g in range(ngrp):
        gsegs = segs[g * per_g:(g + 1) * per_g]
        calls = []
        for (o, off, nblk, inc) in gsegs:
            done = 0
            while done < nblk:
                nb = min(MAXCALL, nblk - done)
                calls.append((o, off + done * P, nb, off // P + done))
                done += nb
        tset = sorted({t for (_, _, _, inc) in gsegs for (_, t, _, _) in inc})
        tiles = []
        for t in tset:
            lst = []
            for (o, off, nblk, inc) in gsegs:
                for (b, tt, a, bb) in inc:
                    if tt == t:
                        lst.append((inc_id, off // P + b, a, bb))
                        inc_id += 1
            tiles.append((t, lst))
        plan.append((calls, tiles))
    return plan, inc_id


def _preprocess(node_ids, edge_index, batch):
    node_ids = np.asarray(node_ids).astype(np.int64)
    src_g = np.asarray(edge_index)[0].astype(np.int64)
    dst_g = np.asarray(edge_index)[1].astype(np.int64)
    batch = np.asarray(batch).astype(np.int64)

    deg = 1.0 + np.bincount(dst_g, minlength=N).astype(np.float64)
    dinv = (1.0 / np.sqrt(deg)).astype(F32)
    selfnorm = (dinv * dinv).astype(F32)

    gcnt = np.bincount(batch, minlength=G)
    goff = np.zeros(G + 1, np.int64)
    goff[1:] = np.cumsum(gcnt)
    node_start = np.array([int(goff[c * GPC]) for c in range(CORES + 1)])
    n_c = node_start[1:] - node_start[:-1]
    TILES = int(np.ceil(n_c.max() / P))
    SLAB = TILES * P
    assert SLAB <= 32767, SLAB
    NGRP = (TILES + GT - 1) // GT

    e_oc = np.searchsorted(node_start[1:], dst_g, side="right")
    e_os = np.searchsorted(node_start[1:], src_g, side="right")

    cores = []
    for c in range(CORES):
        m = e_oc == c
        cores.append(dict(src=src_g[m], dstl=dst_g[m] - node_start[c],
                          own=e_os[m], n=int(n_c[c]), nlo=int(node_start[c])))

    # uniform per-(tile, owner) counts
    K1 = np.zeros((TILES, CORES), np.int64)
    K0 = np.zeros((TILES, 1), np.int64)
    for c in range(CORES):
        t_of = cores[c]["dstl"] // P
        cnt = np.zeros((TILES, CORES), np.int64)
        np.add.at(cnt, (t_of, cores[c]["own"]), 1)
        np.maximum(K1, cnt, out=K1)
        c0 = np.bincount(t_of, minlength=TILES).reshape(-1, 1)
        np.maximum(K0, c0, out=K0)
    K0 = K0 + P  # layer-0 self-loop slots

    segs1, S1 = _build_stream(K1, NGRP, TILES)
    segs0, S0 = _build_stream(K0, NGRP, TILES)
    NB0, NB1 = S0 // P, S1 // P
    plan0, ninc0 = _make_plan(segs0, NGRP, 0)
    plan1, ninc1 = _make_plan(segs1, NGRP, ninc0)

    # graph cover ranges (uniform across cores)
    glo_all = np.stack([goff[c * GPC:(c + 1) * GPC] - node_start[c]
                        for c in range(CORES)])   # [CORES, GPC]
    ghi_all = np.stack([goff[c * GPC + 1:(c + 1) * GPC + 1] - node_start[c]
                        for c in range(CORES)])
    lo_fix = glo_all.min(0)
    hi_fix = ghi_all.max(0)
    MAXCOV = int((hi_fix - lo_fix).max())
    MAXCOV = (MAXCOV + 31) // 32 * 32
    cov_len = np.minimum(MAXCOV, SLAB - lo_fix)

    per_core = []
    for c in range(CORES):
        cd = cores[c]
        t_of = cd["dstl"] // P
        idx0 = np.zeros(S0, np.int16)
        idx1 = np.zeros(S1, np.int16)
        scol0 = np.zeros(S0, np.int64)
        scoef0 = np.zeros(S0, F32)
        scol1 = np.zeros(S1, np.int64)
        scoef1 = np.zeros(S1, F32)

        # --- L123 stream fill ---
        order = np.lexsort((cd["dstl"], t_of, cd["own"], t_of // GT))
        srcs = cd["src"][order]
        dstl = cd["dstl"][order]
        owns = cd["own"][order]
        tof = t_of[order]
        coefs = dinv[srcs] * dinv[dstl + cd["nlo"]]
        ptr = 0
        for (o, off, nblk, inc) in segs1:
            starts = {}
            for (b, t, a, bb) in inc:
                if t not in starts:
                    starts[t] = off + b * P + a
            for t in sorted(starts):
                p = starts[t]
                while ptr < len(srcs) and tof[ptr] == t and owns[ptr] == o:
                    idx1[p] = srcs[ptr] - node_start[o]
                    scol1[p] = dstl[ptr] % P
                    scoef1[p] = coefs[ptr]
                    p += 1
                    ptr += 1
        assert ptr == len(srcs), (c, ptr, len(srcs))

        # --- L0 stream fill (edges + self loops; table = T0 by vocab id) ---
        order0 = np.lexsort((cd["dstl"],))
        s0 = cd["src"][order0]
        d0 = cd["dstl"][order0]
        c0 = dinv[s0] * dinv[d0 + cd["nlo"]]
        ptr = 0
        for (o, off, nblk, inc) in segs0:
            starts = {}
            for (b, t, a, bb) in inc:
                if t not in starts:
                    starts[t] = off + b * P + a
            for t in sorted(starts):
                p = starts[t]
                while ptr < len(s0) and d0[ptr] // P == t:
                    idx0[p] = node_ids[s0[ptr]]
                    scol0[p] = d0[ptr] % P
                    scoef0[p] = c0[ptr]
                    p += 1
                    ptr += 1
                lo, hi = t * P, min((t + 1) * P, cd["n"])
                for dn in range(lo, hi):
                    gid = dn + cd["nlo"]
                    idx0[p] = node_ids[gid]
                    scol0[p] = dn % P
                    scoef0[p] = selfnorm[gid]
                    p += 1
        assert ptr == len(s0)

        seg = np.zeros((ninc1, P, P), F32)
        for plan, scol, scoef in ((plan0, scol0, scoef0),
                                  (plan1, scol1, scoef1)):
            for (calls, tiles) in plan:
                for (t, lst) in tiles:
                    for (inc_id, bglob, a, bb) in lst:
                        rows = np.arange(a, bb)
                        slots = bglob * P + rows
                        seg[inc_id, rows, scol[slots]] = scoef[slots]

        selfn = np.zeros((P, TILES), F32)
        ar = np.arange(cd["n"])
        selfn[ar % P, ar // P] = selfnorm[cd["nlo"]:cd["nlo"] + cd["n"]]

        # graph masks [GPC, P, MAXCOV] bf16 (replicated across partitions)
        mask = np.zeros((GPC, MAXCOV), F32)
        cinv = np.zeros(GPC, F32)
        for j in range(GPC):
            lo = int(glo_all[c, j]) - int(lo_fix[j])
            hi = int(ghi_all[c, j]) - int(lo_fix[j])
            mask[j, lo:hi] = 1.0
            cnt = int(ghi_all[c, j] - glo_all[c, j])
            cinv[j] = 1.0 / max(cnt, 1)
        maskb = np.broadcast_to(mask[:, None, :], (GPC, P, MAXCOV))

        per_core.append(dict(
            idx16=_idx16_layout(np.concatenate([idx0, idx1]).astype(np.int16)),
            seg=np.ascontiguousarray(seg.astype(BF).transpose(1, 0, 2)),
            selfn=selfn,
            mask=np.ascontiguousarray(maskb.astype(BF)),
            cinv=np.broadcast_to(cinv, (P, GPC)).copy(),
        ))

    meta = dict(TILES=TILES, SLAB=SLAB, NGRP=NGRP, NB0=NB0, NB1=NB1,
                S0=S0, S1=S1, segs0=segs0, segs1=segs1,
                plan0=plan0, plan1=plan1, NINC=ninc1,
                lo_fix=lo_fix, cov_len=cov_len, MAXCOV=MAXCOV)
    return per_core, meta


def _build(meta):
    TILES, SLAB, NGRP = meta["TILES"], meta["SLAB"], meta["NGRP"]
    NB0, NB1 = meta["NB0"], meta["NB1"]
    MAXCOV = meta["MAXCOV"]
    f32, bf16, i16 = mybir.dt.float32, mybir.dt.bfloat16, mybir.dt.int16
    AF = mybir.ActivationFunctionType
    nc = bacc.Bacc("TRN2", target_bir_lowering=False, debug=False,
                   num_devices=CORES)

    SCOLS = (meta["S0"] + meta["S1"]) // 16
    idx_in = nc.dram_tensor("idx16", [P, SCOLS], i16, kind="ExternalInput")
    seg_in = nc.dram_tensor("seg", [P, meta["NINC"], P], bf16,
                            kind="ExternalInput")
    selfn_in = nc.dram_tensor("selfn", [P, TILES], f32, kind="ExternalInput")
    mask_in = nc.dram_tensor("mask", [GPC, P, MAXCOV], bf16,
                             kind="ExternalInput")
    cinv_in = nc.dram_tensor("cinv", [P, GPC], f32, kind="ExternalInput")
    embT_in = nc.dram_tensor("embT", [P, 1024], bf16, kind="ExternalInput")
    w0_in = nc.dram_tensor("w0", [P, H], bf16, kind="ExternalInput")
    wl_in = nc.dram_tensor("wl", [P, 3, 2, H], bf16, kind="ExternalInput")
    b_in = nc.dram_tensor("bias", [P, 4, H], f32, kind="ExternalInput")
    cw0_in = nc.dram_tensor("cw0", [P, 4, H], bf16, kind="ExternalInput")
    cb0_in = nc.dram_tensor("cb0", [GPC, H], f32, kind="ExternalInput")
    cw1_in = nc.dram_tensor("cw1", [P, 2, 2], bf16, kind="ExternalInput")
    cb1_in = nc.dram_tensor("cb1", [GPC, 2], f32, kind="ExternalInput")
    out = nc.dram_tensor("out", [GPC, 2], f32, kind="ExternalOutput")

    T0_d = nc.dram_tensor("T0d", [1024, H], bf16)

    with tile.TileContext(nc) as tc:
        with (
            tc.tile_pool(name="const", bufs=1) as cpool,
            tc.tile_pool(name="gat", bufs=10) as gpool,
            tc.tile_pool(name="segp", bufs=4) as segp,
            tc.tile_pool(name="selfp", bufs=4) as selfp,
            tc.tile_pool(name="work", bufs=6) as work,
            tc.tile_pool(name="hmp", bufs=2) as hmp,
            tc.tile_pool(name="ps", bufs=4, space="PSUM") as ps,
            tc.tile_pool(name="pstr", bufs=2, space="PSUM") as pstr,
            tc.tile_pool(name="psmm", bufs=2, space="PSUM") as psmm,
            tc.tile_pool(name="dram", bufs=1, space="DRAM") as dpool,
        ):
            idx_t = cpool.tile([P, SCOLS], i16)
            nc.sync.dma_start(out=idx_t[:], in_=idx_in[:, :])
            selfn_t = cpool.tile([P, TILES], f32)
            nc.sync.dma_start(out=selfn_t[:], in_=selfn_in[:, :])
            cinv_t = cpool.tile([P, GPC], f32)
            nc.sync.dma_start(out=cinv_t[:], in_=cinv_in[:, :])
            embT_t = cpool.tile([P, 1024], bf16)
            nc.sync.dma_start(out=embT_t[:], in_=embT_in[:, :])
            w0_t = cpool.tile([P, H], bf16)
            nc.sync.dma_start(out=w0_t[:], in_=w0_in[:, :])
            wl_t = cpool.tile([P, 3, 2, H], bf16)
            nc.sync.dma_start(out=wl_t[:], in_=wl_in[:, :, :, :])
            bias_t = cpool.tile([P, 4, H], f32)
            nc.sync.dma_start(out=bias_t[:], in_=b_in[:, :, :])

            h_T = nc.alloc_sbuf_tensor("hT", [P, 2, SLAB], bf16)
            ident = cpool.tile([P, P], bf16)
            make_identity(nc, ident[:])

            # ---- T0 = emb @ w0 ----
            for vb in range(8):
                pt0 = psmm.tile([P, H], f32, tag="mm")
                nc.tensor.matmul(out=pt0[:], lhsT=embT_t[:, vb * P:(vb + 1) * P],
                                 rhs=w0_t[:], start=True, stop=True)
                t0s = work.tile([P, H], bf16)
                nc.scalar.copy(out=t0s[:], in_=pt0[:])
                nc.sync.dma_start(out=T0_d[vb * P:(vb + 1) * P, :], in_=t0s[:])

            MB = dpool.tile([SLAB, H], bf16)
            MF = nc.dram_tensor("MFd", [CORES * SLAB, H], bf16,
                                addr_space="Shared")

            def spmm(layer):
                plan = meta["plan0"] if layer == 0 else meta["plan1"]
                base_slot = 0 if layer == 0 else meta["S0"]
                for (calls, tiles) in plan:
                    blk_ref = {}
                    for (o, soff, nb, blk0) in calls:
                        gt_ = gpool.tile([P, MAXCALL, H], bf16)
                        slot0 = base_slot + soff
                        if layer == 0:
                            tab = T0_d[0:1024, :]
                        else:
                            tab = MF[o * SLAB:(o + 1) * SLAB, :]
                        nc.gpsimd.dma_gather(
                            out_ap=gt_[:, 0:nb, :], in_ap=tab,
                            idxs_ap=idx_t[:, slot0 // 16:
                                          (slot0 + nb * P) // 16],
                            num_idxs=nb * P, num_idxs_reg=nb * P,
                            elem_size=H, queue_num=0)
                        for b in range(nb):
                            blk_ref[blk0 + b] = (gt_, b)
                    for (t, lst) in tiles:
                        pt = ps.tile([P, H], f32)
                        if layer > 0:
                            ssrc = selfp.tile([P, H], bf16)
                            nc.sync.dma_start(
                                out=ssrc[:],
                                in_=MB[t * P:(t + 1) * P, :])
                            nc.scalar.activation(
                                out=pt[:], in_=ssrc[:], func=AF.Copy,
                                scale=selfn_t[:, t:t + 1])
                        i0 = lst[0][0]
                        nL = len(lst)
                        st = segp.tile([P, nL, P], bf16, tag="st")
                        nc.scalar.dma_start(out=st[:, :, :],
                                            in_=seg_in[:, i0:i0 + nL, :])
                        for n_i, (inc_id, bglob, a, bb) in enumerate(lst):
                            assert inc_id == i0 + n_i
                            gt_, loc = blk_ref[bglob]
                            nc.tensor.matmul(
                                out=pt[:], lhsT=st[:, n_i, :],
                                rhs=gt_[:, loc, :],
                                start=(layer == 0 and n_i == 0),
                                stop=(n_i == len(lst) - 1),
                                skip_group_check=(layer > 0))
                        tmp = work.tile([P, H], f32)
                        nc.vector.tensor_add(out=tmp[:], in0=pt[:],
                                             in1=bias_t[:, layer, :])
                        tmpb = work.tile([P, H], bf16)
                        nc.scalar.activation(out=tmpb[:], in_=tmp[:],
                                             func=AF.Relu)
                        for fh in range(2):
                            ptr_ = pstr.tile([P, P], bf16)
                            nc.tensor.transpose(
                                out=ptr_[:], in_=tmpb[:, fh * P:(fh + 1) * P],
                                identity=ident[:])
                            nc.vector.tensor_copy(
                                out=h_T[:, fh, t * P:(t + 1) * P], in_=ptr_[:])

            spmm(0)
            for layer in (1, 2, 3):
                # M = h @ W  -> MB, then AllGather -> MF
                for t in range(TILES):
                    pm = psmm.tile([P, H], f32, tag="mm")
                    for fh in range(2):
                        nc.tensor.matmul(
                            out=pm[:],
                            lhsT=h_T[:, fh, t * P:(t + 1) * P],
                            rhs=wl_t[:, layer - 1, fh, :],
                            start=(fh == 0), stop=(fh == 1))
                    mt = work.tile([P, H], bf16)
                    nc.scalar.copy(out=mt[:], in_=pm[:])
                    nc.sync.dma_start(out=MB[t * P:(t + 1) * P, :], in_=mt[:])
                nc.gpsimd.collective_compute(
                    "AllGather", mybir.AluOpType.bypass,
                    replica_groups=[list(range(CORES))],
                    ins=[MB[:].opt()], outs=[MF[:].opt()])
                spmm(layer)

            # ---- pooling (masked mean/max on h_T) ----
            pooled = []
            for fh in range(2):
                mean_t = cpool.tile([P, GPC], f32, tag=f"mean{fh}")
                max_t = cpool.tile([P, GPC], f32, tag=f"max{fh}")
                nc.vector.memset(mean_t[:], 0.0)
                nc.vector.memset(max_t[:], 0.0)
                pooled.append((mean_t, max_t))
            lo_fix, cov_len = meta["lo_fix"], meta["cov_len"]
            for j in range(GPC):
                mk = hmp.tile([P, MAXCOV], bf16, tag="mask")
                ln = int(cov_len[j])
                nc.sync.dma_start(out=mk[:, 0:ln], in_=mask_in[j, :, 0:ln])
                for fh in range(2):
                    hm = hmp.tile([P, MAXCOV], f32, tag="hm")
                    lo = int(lo_fix[j])
                    nc.vector.tensor_mul(out=hm[:, 0:ln],
                                         in0=h_T[:, fh, lo:lo + ln],
                                         in1=mk[:, 0:ln])
                    nc.vector.tensor_reduce(
                        out=pooled[fh][0][:, j:j + 1], in_=hm[:, 0:ln],
                        axis=mybir.AxisListType.X, op=mybir.AluOpType.add)
                    nc.vector.tensor_reduce(
                        out=pooled[fh][1][:, j:j + 1], in_=hm[:, 0:ln],
                        axis=mybir.AxisListType.X, op=mybir.AluOpType.max)
            # scale means by 1/cnt, cast to bf16 lhsT chunks
            chunks = []
            for fh in range(2):
                mean_t, max_t = pooled[fh]
                nc.vector.tensor_mul(out=mean_t[:], in0=mean_t[:],
                                     in1=cinv_t[:])
            for (kind, fh) in ((0, 0), (0, 1), (1, 0), (1, 1)):
                src = pooled[fh][kind]
                cb = work.tile([P, GPC], bf16, tag=f"ch{kind}{fh}")
                nc.vector.tensor_copy(out=cb[:], in_=src[:])
                chunks.append(cb)

            # ---- classifier MLP ----
            cw0_t = cpool.tile([P, 4, H], bf16)
            nc.sync.dma_start(out=cw0_t[:], in_=cw0_in[:, :, :])
            cb0_t = cpool.tile([GPC, H], f32)
            nc.sync.dma_start(out=cb0_t[:], in_=cb0_in[:, :])
            cw1_t = cpool.tile([P, 2, 2], bf16)
            nc.sync.dma_start(out=cw1_t[:], in_=cw1_in[:, :, :])
            cb1_t = cpool.tile([GPC, 2], f32)
            nc.sync.dma_start(out=cb1_t[:], in_=cb1_in[:, :])

            ph_full = psmm.tile([P, H], f32, tag="mm")
            ph = ph_full[0:GPC, :]
            for k in range(4):
                nc.tensor.matmul(out=ph[:], lhsT=chunks[k][:],
                                 rhs=cw0_t[:, k, :],
                                 start=(k == 0), stop=(k == 3))
            hc1 = work.tile([GPC, H], f32, tag="hc1")
            nc.vector.tensor_add(out=hc1[:], in0=ph[:], in1=cb0_t[:])
            hcb = work.tile([GPC, H], bf16, tag="hcb")
            nc.scalar.activation(out=hcb[:], in_=hc1[:], func=AF.Relu)
            hTt = []
            for k in range(2):
                ptr_ = pstr.tile([P, P], bf16)
                nc.tensor.transpose(out=ptr_[0:P, 0:GPC],
                                    in_=hcb[:, k * P:(k + 1) * P],
                                    identity=ident[0:GPC, 0:GPC])
                ht = work.tile([P, GPC], bf16, tag=f"hTt{k}")
                nc.vector.tensor_copy(out=ht[:], in_=ptr_[0:P, 0:GPC])
                hTt.append(ht)
            pl_full = psmm.tile([P, H], f32, tag="mm")
            pl = pl_full[0:GPC, 0:2]
            for k in range(2):
                nc.tensor.matmul(out=pl[:], lhsT=hTt[k][:],
                                 rhs=cw1_t[:, k, :],
                                 start=(k == 0), stop=(k == 1))
            lg = work.tile([GPC, 2], f32, tag="lg")
            nc.vector.tensor_add(out=lg[:], in0=pl[:], in1=cb1_t[:])
            nc.sync.dma_start(out=out[:, :], in_=lg[:])
    nc.finalize()
    return nc


def kernel(node_ids, edge_index, batch, emb, w0, b0, w1, b1, w2, b2, w3, b3,
           cw0, cb0, cw1, cb1):
    per_core, meta = _preprocess(node_ids, edge_index, batch)
    nc = _build(meta)

    embT = np.zeros((P, 1024), F32)
    embT[:, :V] = np.asarray(emb, F32).T
    wlk = np.transpose(np.stack([np.asarray(w, F32).reshape(2, P, H)
                    for w in (w1, w2, w3)]), (2, 0, 1, 3)).copy()
    biases = np.stack([np.broadcast_to(np.asarray(b, F32), (P, H))
                       for b in (b0, b1, b2, b3)], axis=1).copy()
    ins = []
    for c in range(CORES):
        pc = per_core[c]
        ins.append(dict(
            idx16=pc["idx16"], seg=pc["seg"], selfn=pc["selfn"],
            mask=pc["mask"], cinv=pc["cinv"],
            embT=embT.astype(BF), w0=np.asarray(w0, F32).astype(BF),
            wl=wlk.astype(BF), bias=biases.astype(F32),
            cw0=np.transpose(np.asarray(cw0, F32).reshape(4, P, H), (1, 0, 2)).astype(BF),
            cb0=np.broadcast_to(np.asarray(cb0, F32), (GPC, H)).copy(),
            cw1=np.transpose(np.asarray(cw1, F32).reshape(2, P, 2), (1, 0, 2)).astype(BF),
            cb1=np.broadcast_to(np.asarray(cb1, F32), (GPC, 2)).copy(),
        ))
    trace = False
    try:  # register NTFF hook so exec_time_ns is measurable (best effort)
        import sys, types
        import antenv
        if "antenv.axon_hooks" not in sys.modules:
            hooks = types.ModuleType("antenv.axon_hooks")
            hooks._h = None
            hooks.set_axon_ntff_profile_hook = lambda h: setattr(hooks, "_h", h)
            hooks.get_axon_ntff_profile_hook = lambda: hooks._h
            sys.modules["antenv.axon_hooks"] = hooks
            antenv.axon_hooks = hooks
            from trn_agent_boot.trn_boot import _ntff_profile_via_ctypes
            hk = _ntff_profile_via_ctypes("/opt/axon/libaxon_pjrt.so")
            if hk is not None:
                hooks.set_axon_ntff_profile_hook(hk)
                trace = True
        else:
            trace = True
    except Exception:
        trace = False
    res = run_bass_kernel_spmd(nc, ins, core_ids=list(range(CORES)),
                               trace=trace)
    logits = np.concatenate([res.results[c]["out"] for c in range(CORES)], 0)
    globals()["last_exec_ns"] = res.exec_time_ns
    globals()["last_res"] = res
    return logits.astype(np.float32)



# revision 25
# speedup vs baseline: 2.0564x; 1.1758x over previous
"""GCN (EnhancedTaintFlowGNN) on 8 Trainium2 NeuronCores.

Sharding: 32 graphs/core (batch sorted -> contiguous node ranges). Per GCN
layer: M = h @ W locally (PE), AllGather M, pull-side SpMM via dma_gather
from 8 owner-aligned int16 tables + PE segment-matmuls (norm folded into the
segment coefficients). Self-loops preloaded into PSUM by ACT scaled by
dinv^2. Layer 0 gathers rows of T0 = emb @ w0 (vocab table, replicated on
every core) so no exchange is needed. Pooling: masked DVE mean/max on
transposed features; classifier MLP on PE.
"""
import numpy as np
import ml_dtypes

import concourse.bass as bass
import concourse.bacc as bacc
import concourse.tile as tile
from concourse import mybir
from concourse.bass_utils import run_bass_kernel_spmd
from concourse.masks import make_identity

P = 128
N, E, G, V, D, H = 150000, 300000, 256, 1000, 128, 256
CORES = 8
GPC = G // CORES
GT = 16        # dst-tiles per psum group
MAXCALL = 8    # max 128-slot blocks per dma_gather call
BF = ml_dtypes.bfloat16
F32 = np.float32
PREPARE = False  # use prepare_only + trigger_dma for gathers


def _idx16_layout(idx):
    n = idx.shape[0]
    s = n // 16
    out = np.zeros((P, s), np.int16)
    blk = idx.reshape(s, 16).T.astype(np.int16)
    for g in range(8):
        out[g * 16:(g + 1) * 16, :] = blk
    return out


def _build_stream(counts2d, ngrp, tiles):
    """counts2d [tiles, n_own]. Returns (segs, total_slots).
    segs: per (group, owner): (owner, slot_off, nblk, incidences)
    incidences: (local_blk, tile, a, b) row-ranges inside each block."""
    n_own = counts2d.shape[1]
    segs = []
    off = 0
    for g in range(ngrp):
        tl = list(range(g * GT, min((g + 1) * GT, tiles)))
        for o in range(n_own):
            cnts = [int(counts2d[t, o]) for t in tl]
            seglen = sum(cnts)
            nblk = max(1, (seglen + P - 1) // P)
            inc = []
            r = 0
            for t, k in zip(tl, cnts):
                if k == 0:
                    continue
                r0, r1 = r, r + k
                for b in range(r0 // P, (r1 - 1) // P + 1):
                    a = max(r0 - b * P, 0)
                    bb = min(r1 - b * P, P)
                    if bb > a:
                        inc.append((b, t, a, bb))
                r = r1
            segs.append((o, off, nblk, inc))
            off += nblk * P
    return segs, off


def _make_plan(segs, ngrp, inc_base):
    """Per group: calls [(owner, slot_off, nblk, blk0)] and tiles
    [(t, [(inc_id, bglob)])]; inc ids sequential in builder order."""
    per_g = len(segs) // ngrp
    plan = []
    inc_id = inc_base
    for g in range(ngrp):
        gsegs = segs[g * per_g:(g + 1) * per_g]
        calls = []
        for (o, off, nblk, inc) in gsegs:
            done = 0
            while done < nblk:
                nb = min(MAXCALL, nblk - done)
                calls.append((o, off + done * P, nb, off // P + done))
                done += nb
        tset = sorted({t for (_, _, _, inc) in gsegs for (_, t, _, _) in inc})
        tiles = []
        for t in tset:
            lst = []
            for (o, off, nblk, inc) in gsegs:
                for (b, tt, a, bb) in inc:
                    if tt == t:
                        lst.append((inc_id, off // P + b, a, bb))
                        inc_id += 1
            tiles.append((t, lst))
        plan.append((calls, tiles))
    return plan, inc_id


def _preprocess(node_ids, edge_index, batch, emb_np):
    node_ids = np.asarray(node_ids).astype(np.int64)
    src_g = np.asarray(edge_index)[0].astype(np.int64)
    dst_g = np.asarray(edge_index)[1].astype(np.int64)
    batch = np.asarray(batch).astype(np.int64)

    deg = 1.0 + np.bincount(dst_g, minlength=N).astype(np.float64)
    dinv = (1.0 / np.sqrt(deg)).astype(F32)
    selfnorm = (dinv * dinv).astype(F32)

    gcnt = np.bincount(batch, minlength=G)
    goff = np.zeros(G + 1, np.int64)
    goff[1:] = np.cumsum(gcnt)
    node_start = np.array([int(goff[c * GPC]) for c in range(CORES + 1)])
    n_c = node_start[1:] - node_start[:-1]
    TILES = int(np.ceil(n_c.max() / P))
    SLAB = TILES * P
    assert SLAB <= 32767, SLAB
    NGRP = (TILES + GT - 1) // GT

    e_oc = np.searchsorted(node_start[1:], dst_g, side="right")
    e_os = np.searchsorted(node_start[1:], src_g, side="right")

    cores = []
    for c in range(CORES):
        m = e_oc == c
        cores.append(dict(src=src_g[m], dstl=dst_g[m] - node_start[c],
                          own=e_os[m], n=int(n_c[c]), nlo=int(node_start[c])))

    # uniform per-(tile, owner) counts
    K1 = np.zeros((TILES, CORES), np.int64)
    K0 = np.zeros((TILES, 1), np.int64)
    for c in range(CORES):
        t_of = cores[c]["dstl"] // P
        cnt = np.zeros((TILES, CORES), np.int64)
        np.add.at(cnt, (t_of, cores[c]["own"]), 1)
        np.maximum(K1, cnt, out=K1)
        c0 = np.bincount(t_of, minlength=TILES).reshape(-1, 1)
        np.maximum(K0, c0, out=K0)

    segs1, S1 = _build_stream(K1, NGRP, TILES)
    segs0, S0 = _build_stream(K0, NGRP, TILES)
    NB0, NB1 = S0 // P, S1 // P
    plan0, ninc0 = _make_plan(segs0, NGRP, 0)
    plan1, ninc1 = _make_plan(segs1, NGRP, ninc0)

    # graph cover ranges (uniform across cores)
    glo_all = np.stack([goff[c * GPC:(c + 1) * GPC] - node_start[c]
                        for c in range(CORES)])   # [CORES, GPC]
    ghi_all = np.stack([goff[c * GPC + 1:(c + 1) * GPC + 1] - node_start[c]
                        for c in range(CORES)])
    lo_fix = glo_all.min(0)
    hi_fix = ghi_all.max(0)
    MAXCOV = int((hi_fix - lo_fix).max())
    MAXCOV = (MAXCOV + 31) // 32 * 32
    cov_len = np.minimum(MAXCOV, SLAB - lo_fix)

    per_core = []
    for c in range(CORES):
        cd = cores[c]
        t_of = cd["dstl"] // P
        idx0 = np.zeros(S0, np.int16)
        idx1 = np.zeros(S1, np.int16)
        scol0 = np.zeros(S0, np.int64)
        scoef0 = np.zeros(S0, F32)
        scol1 = np.zeros(S1, np.int64)
        scoef1 = np.zeros(S1, F32)

        # --- L123 stream fill ---
        order = np.lexsort((cd["dstl"], t_of, cd["own"], t_of // GT))
        srcs = cd["src"][order]
        dstl = cd["dstl"][order]
        owns = cd["own"][order]
        tof = t_of[order]
        coefs = dinv[srcs] * dinv[dstl + cd["nlo"]]
        ptr = 0
        for (o, off, nblk, inc) in segs1:
            starts = {}
            for (b, t, a, bb) in inc:
                if t not in starts:
                    starts[t] = off + b * P + a
            for t in sorted(starts):
                p = starts[t]
                while ptr < len(srcs) and tof[ptr] == t and owns[ptr] == o:
                    idx1[p] = srcs[ptr] - node_start[o]
                    scol1[p] = dstl[ptr] % P
                    scoef1[p] = coefs[ptr]
                    p += 1
                    ptr += 1
        assert ptr == len(srcs), (c, ptr, len(srcs))

        # --- L0 stream fill (edges only; table = T0 by vocab id) ---
        order0 = np.lexsort((cd["dstl"],))
        s0 = cd["src"][order0]
        d0 = cd["dstl"][order0]
        c0 = dinv[s0] * dinv[d0 + cd["nlo"]]
        ptr = 0
        for (o, off, nblk, inc) in segs0:
            starts = {}
            for (b, t, a, bb) in inc:
                if t not in starts:
                    starts[t] = off + b * P + a
            for t in sorted(starts):
                p = starts[t]
                while ptr < len(s0) and d0[ptr] // P == t:
                    idx0[p] = node_ids[s0[ptr]]
                    scol0[p] = d0[ptr] % P
                    scoef0[p] = c0[ptr]
                    p += 1
                    ptr += 1
        assert ptr == len(s0)

        # self-loop term via host-precomputed scaled embedding rows
        xs = np.zeros((SLAB, D), F32)
        ids_own = node_ids[cd["nlo"]:cd["nlo"] + cd["n"]]
        xs[:cd["n"], :] = emb_np[ids_own, :] * \
            selfnorm[cd["nlo"]:cd["nlo"] + cd["n"], None]
        xselfT = np.ascontiguousarray(xs.T.astype(BF))

        seg = np.zeros((ninc1, P, P), F32)
        for plan, scol, scoef in ((plan0, scol0, scoef0),
                                  (plan1, scol1, scoef1)):
            for (calls, tiles) in plan:
                for (t, lst) in tiles:
                    for (inc_id, bglob, a, bb) in lst:
                        rows = np.arange(a, bb)
                        slots = bglob * P + rows
                        seg[inc_id, rows, scol[slots]] = scoef[slots]

        selfn = np.zeros((P, TILES), F32)
        ar = np.arange(cd["n"])
        selfn[ar % P, ar // P] = selfnorm[cd["nlo"]:cd["nlo"] + cd["n"]]

        # graph masks [GPC, P, MAXCOV] bf16 (replicated across partitions)
        mask = np.zeros((GPC, MAXCOV), F32)
        cinv = np.zeros(GPC, F32)
        for j in range(GPC):
            lo = int(glo_all[c, j]) - int(lo_fix[j])
            hi = int(ghi_all[c, j]) - int(lo_fix[j])
            mask[j, lo:hi] = 1.0
            cnt = int(ghi_all[c, j] - glo_all[c, j])
            cinv[j] = 1.0 / max(cnt, 1)
        maskb = np.broadcast_to(mask[:, None, :], (GPC, P, MAXCOV))

        per_core.append(dict(
            idx16=_idx16_layout(np.concatenate([idx0, idx1]).astype(np.int16)),
            seg=np.ascontiguousarray(seg.astype(BF).transpose(1, 0, 2)),
            xselfT=xselfT,
            selfn=selfn,
            mask=np.ascontiguousarray(maskb.astype(BF)),
            cinv=np.broadcast_to(cinv, (P, GPC)).copy(),
        ))

    meta = dict(TILES=TILES, SLAB=SLAB, NGRP=NGRP, NB0=NB0, NB1=NB1,
                S0=S0, S1=S1, segs0=segs0, segs1=segs1,
                plan0=plan0, plan1=plan1, NINC=ninc1,
                lo_fix=lo_fix, cov_len=cov_len, MAXCOV=MAXCOV)
    return per_core, meta


def _build(meta):
    TILES, SLAB, NGRP = meta["TILES"], meta["SLAB"], meta["NGRP"]
    NB0, NB1 = meta["NB0"], meta["NB1"]
    MAXCOV = meta["MAXCOV"]
    f32, bf16, i16 = mybir.dt.float32, mybir.dt.bfloat16, mybir.dt.int16
    AF = mybir.ActivationFunctionType
    nc = bacc.Bacc("TRN2", target_bir_lowering=False, debug=False,
                   num_devices=CORES)

    SCOLS = (meta["S0"] + meta["S1"]) // 16
    idx_in = nc.dram_tensor("idx16", [P, SCOLS], i16, kind="ExternalInput")
    seg_in = nc.dram_tensor("seg", [P, meta["NINC"], P], bf16,
                            kind="ExternalInput")
    selfn_in = nc.dram_tensor("selfn", [P, TILES], f32, kind="ExternalInput")
    xselfT_in = nc.dram_tensor("xselfT", [P, SLAB], bf16,
                               kind="ExternalInput")
    mask_in = nc.dram_tensor("mask", [GPC, P, MAXCOV], bf16,
                             kind="ExternalInput")
    cinv_in = nc.dram_tensor("cinv", [P, GPC], f32, kind="ExternalInput")
    embT_in = nc.dram_tensor("embT", [P, 1024], bf16, kind="ExternalInput")
    w0_in = nc.dram_tensor("w0", [P, H], bf16, kind="ExternalInput")
    wl_in = nc.dram_tensor("wl", [P, 3, 2, H], bf16, kind="ExternalInput")
    b_in = nc.dram_tensor("bias", [P, 4, H], f32, kind="ExternalInput")
    cw0_in = nc.dram_tensor("cw0", [P, 4, H], bf16, kind="ExternalInput")
    cb0_in = nc.dram_tensor("cb0", [GPC, H], f32, kind="ExternalInput")
    cw1_in = nc.dram_tensor("cw1", [P, 2, 2], bf16, kind="ExternalInput")
    cb1_in = nc.dram_tensor("cb1", [GPC, 2], f32, kind="ExternalInput")
    out = nc.dram_tensor("out", [GPC, 2], f32, kind="ExternalOutput")

    T0_d = nc.dram_tensor("T0d", [1024, H], bf16)

    with tile.TileContext(nc) as tc:
        with (
            tc.tile_pool(name="const", bufs=1) as cpool,
            tc.tile_pool(name="gat", bufs=3) as gpool,
            tc.tile_pool(name="segp", bufs=3) as segp,
            tc.tile_pool(name="selfp", bufs=4) as selfp,
            tc.tile_pool(name="work", bufs=4) as work,
            tc.tile_pool(name="hmp", bufs=2) as hmp,
            tc.tile_pool(name="ps", bufs=4, space="PSUM") as ps,
            tc.tile_pool(name="pstr", bufs=2, space="PSUM") as pstr,
            tc.tile_pool(name="psmm", bufs=2, space="PSUM") as psmm,
            tc.tile_pool(name="dram", bufs=1, space="DRAM") as dpool,
        ):
            idx_t = cpool.tile([P, SCOLS], i16)
            nc.sync.dma_start(out=idx_t[:], in_=idx_in[:, :])
            selfn_t = cpool.tile([P, TILES], f32)
            nc.sync.dma_start(out=selfn_t[:], in_=selfn_in[:, :])
            cinv_t = cpool.tile([P, GPC], f32)
            nc.sync.dma_start(out=cinv_t[:], in_=cinv_in[:, :])
            embT_t = cpool.tile([P, 1024], bf16)
            nc.sync.dma_start(out=embT_t[:], in_=embT_in[:, :])
            w0_t = cpool.tile([P, H], bf16)
            nc.sync.dma_start(out=w0_t[:], in_=w0_in[:, :])
            wl_t = cpool.tile([P, 3, 2, H], bf16)
            nc.sync.dma_start(out=wl_t[:], in_=wl_in[:, :, :, :])
            bias_t = cpool.tile([P, 4, H], f32)
            nc.sync.dma_start(out=bias_t[:], in_=b_in[:, :, :])

            h_T = nc.alloc_sbuf_tensor("hT", [P, 2, SLAB], bf16)
            ident = cpool.tile([P, P], bf16)
            make_identity(nc, ident[:])

            # ---- T0 = emb @ w0 ----
            for vb in range(8):
                pt0 = psmm.tile([P, H], f32, tag="mm")
                nc.tensor.matmul(out=pt0[:], lhsT=embT_t[:, vb * P:(vb + 1) * P],
                                 rhs=w0_t[:], start=True, stop=True)
                t0s = work.tile([P, H], bf16)
                nc.scalar.copy(out=t0s[:], in_=pt0[:])
                nc.sync.dma_start(out=T0_d[vb * P:(vb + 1) * P, :], in_=t0s[:])

            MB = dpool.tile([SLAB, H], bf16)
            MF = nc.dram_tensor("MFd", [CORES * SLAB, H], bf16,
                                addr_space="Shared")

            dma_sem = nc.alloc_semaphore("swdge_dma")

            def gmeta(plan):
                out = []
                for (calls, tiles) in plan:
                    b0 = min(c[3] for c in calls)
                    nb = sum(c[2] for c in calls)
                    out.append((b0, nb))
                return out
            gm0, gm1 = gmeta(meta["plan0"]), gmeta(meta["plan1"])
            NBGMAX = max(nb for (_, nb) in gm0 + gm1)
            PF = 3  # gather group prefetch depth == gpool bufs

            def spmm(layer):
                plan = meta["plan0"] if layer == 0 else meta["plan1"]
                gmv = gm0 if layer == 0 else gm1
                base_slot = 0 if layer == 0 else meta["S0"]
                NG = len(plan)
                gt_map = {}

                def prep(g):
                    calls, _ = plan[g]
                    b0, _ = gmv[g]
                    gt_ = gpool.tile([P, NBGMAX, H], bf16, tag="gtg")
                    gt_map[g] = (gt_, b0)
                    for (o, soff, nb, blk0) in calls:
                        slot0 = base_slot + soff
                        if layer == 0:
                            tab = T0_d[0:1024, :]
                        else:
                            tab = MF[o * SLAB:(o + 1) * SLAB, :]
                        nc.gpsimd.dma_gather(
                            out_ap=gt_[:, blk0 - b0:blk0 - b0 + nb, :],
                            in_ap=tab,
                            idxs_ap=idx_t[:, slot0 // 16:
                                          (slot0 + nb * P) // 16],
                            num_idxs=nb * P, num_idxs_reg=nb * P,
                            elem_size=H, queue_num=0,
                            prepare_only=PREPARE, sem=dma_sem if PREPARE
                            else None)
                    if PREPARE:
                        nc.gpsimd.trigger_dma(count=None)

                for g in range(min(PF, NG)):
                    prep(g)
                for g, (calls, tiles) in enumerate(plan):
                    gt_, b0 = gt_map.pop(g)
                    tmap = dict(tiles)
                    for t in range(g * GT, min((g + 1) * GT, TILES)):
                        lst = tmap.get(t, [])
                        nL = len(lst)
                        pt = ps.tile([P, H], f32)
                        if layer > 0:
                            ssrc = selfp.tile([P, H], bf16, tag="ss")
                            nc.sync.dma_start(
                                out=ssrc[:],
                                in_=MB[t * P:(t + 1) * P, :])
                            nc.scalar.activation(
                                out=pt[:], in_=ssrc[:], func=AF.Copy,
                                scale=selfn_t[:, t:t + 1])
                        else:
                            xs = selfp.tile([P, P], bf16, tag="xs")
                            nc.sync.dma_start(
                                out=xs[:],
                                in_=xselfT_in[:, t * P:(t + 1) * P])
                            nc.tensor.matmul(
                                out=pt[:], lhsT=xs[:], rhs=w0_t[:],
                                start=True, stop=(nL == 0))
                        if nL:
                            i0 = lst[0][0]
                            st = segp.tile([P, nL, P], bf16, tag="st")
                            nc.scalar.dma_start(out=st[:, :, :],
                                                in_=seg_in[:, i0:i0 + nL, :])
                        for n_i, (inc_id, bglob, a, bb) in enumerate(lst):
                            nc.tensor.matmul(
                                out=pt[:], lhsT=st[:, n_i, :],
                                rhs=gt_[:, bglob - b0, :],
                                start=False,
                                stop=(n_i == nL - 1),
                                skip_group_check=(layer > 0))
                        tmp = work.tile([P, H], f32)
                        nc.vector.tensor_add(out=tmp[:], in0=pt[:],
                                             in1=bias_t[:, layer, :])
                        tmpb = work.tile([P, H], bf16)
                        nc.scalar.activation(out=tmpb[:], in_=tmp[:],
                                             func=AF.Relu)
                        for fh in range(2):
                            ptr_ = pstr.tile([P, P], bf16)
                            nc.tensor.transpose(
                                out=ptr_[:], in_=tmpb[:, fh * P:(fh + 1) * P],
                                identity=ident[:])
                            nc.vector.tensor_copy(
                                out=h_T[:, fh, t * P:(t + 1) * P], in_=ptr_[:])
                    if g + PF < NG:
                        prep(g + PF)

            spmm(0)
            for layer in (1, 2, 3):
                # M = h @ W  -> MB, then AllGather -> MF
                for t in range(TILES):
                    pm = psmm.tile([P, H], f32, tag="mm")
                    for fh in range(2):
                        nc.tensor.matmul(
                            out=pm[:],
                            lhsT=h_T[:, fh, t * P:(t + 1) * P],
                            rhs=wl_t[:, layer - 1, fh, :],
                            start=(fh == 0), stop=(fh == 1))
                    mt = work.tile([P, H], bf16)
                    nc.scalar.copy(out=mt[:], in_=pm[:])
                    nc.sync.dma_start(out=MB[t * P:(t + 1) * P, :], in_=mt[:])
                nc.gpsimd.collective_compute(
                    "AllGather", mybir.AluOpType.bypass,
                    replica_groups=[list(range(CORES))],
                    ins=[MB[:].opt()], outs=[MF[:].opt()])
                spmm(layer)

            # ---- pooling (masked mean/max on h_T) ----
            pooled = []
            for fh in range(2):
                mean_t = cpool.tile([P, GPC], f32, tag=f"mean{fh}")
                max_t = cpool.tile([P, GPC], f32, tag=f"max{fh}")
                nc.vector.memset(mean_t[:], 0.0)
                nc.vector.memset(max_t[:], 0.0)
                pooled.append((mean_t, max_t))
            lo_fix, cov_len = meta["lo_fix"], meta["cov_len"]
            for j in range(GPC):
                ln = int(cov_len[j])
                lo = int(lo_fix[j])
                mka = hmp.tile([P, MAXCOV], bf16, tag="mask")
                mkb = hmp.tile([P, MAXCOV], bf16, tag="mask")
                mks = [mka, mkb]
                nc.sync.dma_start(out=mks[0][:, 0:ln],
                                  in_=mask_in[j, :, 0:ln])
                nc.vector.tensor_copy(out=mks[1][:, 0:ln],
                                      in_=mks[0][:, 0:ln])
                for fh in range(2):
                    mk = mks[fh]
                    nc.vector.tensor_mul(out=mk[:, 0:ln],
                                         in0=h_T[:, fh, lo:lo + ln],
                                         in1=mk[:, 0:ln])
                    nc.vector.tensor_reduce(
                        out=pooled[fh][0][:, j:j + 1], in_=mk[:, 0:ln],
                        axis=mybir.AxisListType.X, op=mybir.AluOpType.add)
                    nc.vector.tensor_reduce(
                        out=pooled[fh][1][:, j:j + 1], in_=mk[:, 0:ln],
                        axis=mybir.AxisListType.X, op=mybir.AluOpType.max)
            # scale means by 1/cnt, cast to bf16 lhsT chunks
            chunks = []
            for fh in range(2):
                mean_t, max_t = pooled[fh]
                nc.vector.tensor_mul(out=mean_t[:], in0=mean_t[:],
                                     in1=cinv_t[:])
            for (kind, fh) in ((0, 0), (0, 1), (1, 0), (1, 1)):
                src = pooled[fh][kind]
                cb = work.tile([P, GPC], bf16, tag=f"ch{kind}{fh}")
                nc.vector.tensor_copy(out=cb[:], in_=src[:])
                chunks.append(cb)

            # ---- classifier MLP ----
            cw0_t = cpool.tile([P, 4, H], bf16)
            nc.sync.dma_start(out=cw0_t[:], in_=cw0_in[:, :, :])
            cb0_t = cpool.tile([GPC, H], f32)
            nc.sync.dma_start(out=cb0_t[:], in_=cb0_in[:, :])
            cw1_t = cpool.tile([P, 2, 2], bf16)
            nc.sync.dma_start(out=cw1_t[:], in_=cw1_in[:, :, :])
            cb1_t = cpool.tile([GPC, 2], f32)
            nc.sync.dma_start(out=cb1_t[:], in_=cb1_in[:, :])

            ph_full = psmm.tile([P, H], f32, tag="mm")
            ph = ph_full[0:GPC, :]
            for k in range(4):
                nc.tensor.matmul(out=ph[:], lhsT=chunks[k][:],
                                 rhs=cw0_t[:, k, :],
                                 start=(k == 0), stop=(k == 3))
            hc1 = work.tile([GPC, H], f32, tag="hc1")
            nc.vector.tensor_add(out=hc1[:], in0=ph[:], in1=cb0_t[:])
            hcb = work.tile([GPC, H], bf16, tag="hcb")
            nc.scalar.activation(out=hcb[:], in_=hc1[:], func=AF.Relu)
            hTt = []
            for k in range(2):
                ptr_ = pstr.tile([P, P], bf16)
                nc.tensor.transpose(out=ptr_[0:P, 0:GPC],
                                    in_=hcb[:, k * P:(k + 1) * P],
                                    identity=ident[0:GPC, 0:GPC])
                ht = work.tile([P, GPC], bf16, tag=f"hTt{k}")
                nc.vector.tensor_copy(out=ht[:], in_=ptr_[0:P, 0:GPC])
                hTt.append(ht)
            pl_full = psmm.tile([P, H], f32, tag="mm")
            pl = pl_full[0:GPC, 0:2]
            for k in range(2):
                nc.tensor.matmul(out=pl[:], lhsT=hTt[k][:],
                                 rhs=cw1_t[:, k, :],
                                 start=(k == 0), stop=(k == 1))
            lg = work.tile([GPC, 2], f32, tag="lg")
            nc.vector.tensor_add(out=lg[:], in0=pl[:], in1=cb1_t[:])
            nc.sync.dma_start(out=out[:, :], in_=lg[:])
    nc.finalize()
    return nc


def kernel(node_ids, edge_index, batch, emb, w0, b0, w1, b1, w2, b2, w3, b3,
           cw0, cb0, cw1, cb1):
    per_core, meta = _preprocess(node_ids, edge_index, batch,
                                 np.asarray(emb, F32))
    nc = _build(meta)

    embT = np.zeros((P, 1024), F32)
    embT[:, :V] = np.asarray(emb, F32).T
    wlk = np.transpose(np.stack([np.asarray(w, F32).reshape(2, P, H)
                    for w in (w1, w2, w3)]), (2, 0, 1, 3)).copy()
    biases = np.stack([np.broadcast_to(np.asarray(b, F32), (P, H))
                       for b in (b0, b1, b2, b3)], axis=1).copy()
    ins = []
    for c in range(CORES):
        pc = per_core[c]
        ins.append(dict(
            idx16=pc["idx16"], seg=pc["seg"], selfn=pc["selfn"],
            xselfT=pc["xselfT"], mask=pc["mask"], cinv=pc["cinv"],
            embT=embT.astype(BF), w0=np.asarray(w0, F32).astype(BF),
            wl=wlk.astype(BF), bias=biases.astype(F32),
            cw0=np.transpose(np.asarray(cw0, F32).reshape(4, P, H), (1, 0, 2)).astype(BF),
            cb0=np.broadcast_to(np.asarray(cb0, F32), (GPC, H)).copy(),
            cw1=np.transpose(np.asarray(cw1, F32).reshape(2, P, 2), (1, 0, 2)).astype(BF),
            cb1=np.broadcast_to(np.asarray(cb1, F32), (GPC, 2)).copy(),
        ))
    trace = False
    try:  # register NTFF hook so exec_time_ns is measurable (best effort)
        import sys, types
        import antenv
        if "antenv.axon_hooks" not in sys.modules:
            hooks = types.ModuleType("antenv.axon_hooks")
            hooks._h = None
            hooks.set_axon_ntff_profile_hook = lambda h: setattr(hooks, "_h", h)
            hooks.get_axon_ntff_profile_hook = lambda: hooks._h
            sys.modules["antenv.axon_hooks"] = hooks
            antenv.axon_hooks = hooks
            from trn_agent_boot.trn_boot import _ntff_profile_via_ctypes
            hk = _ntff_profile_via_ctypes("/opt/axon/libaxon_pjrt.so")
            if hk is not None:
                hooks.set_axon_ntff_profile_hook(hk)
                trace = True
        else:
            trace = True
    except Exception:
        trace = False
    res = run_bass_kernel_spmd(nc, ins, core_ids=list(range(CORES)),
                               trace=trace)
    logits = np.concatenate([res.results[c]["out"] for c in range(CORES)], 0)
    globals()["last_exec_ns"] = res.exec_time_ns
    globals()["last_res"] = res
    return logits.astype(np.float32)



# revision 55
# speedup vs baseline: 2.1309x; 1.0362x over previous
"""GCN (EnhancedTaintFlowGNN) on 8 Trainium2 NeuronCores.

Sharding: 32 graphs/core (batch sorted -> contiguous node ranges). Per GCN
layer: M = h @ W locally (PE), AllGather M, pull-side SpMM via dma_gather
from 8 owner-aligned int16 tables + PE segment-matmuls (norm folded into the
segment coefficients). Self-loops preloaded into PSUM by ACT scaled by
dinv^2. Layer 0 gathers rows of T0 = emb @ w0 (vocab table, replicated on
every core) so no exchange is needed. Pooling: masked DVE mean/max on
transposed features; classifier MLP on PE.
"""
import numpy as np
import ml_dtypes

import concourse.bass as bass
import concourse.bacc as bacc
import concourse.tile as tile
from concourse import mybir
from concourse.bass_utils import run_bass_kernel_spmd
from concourse.masks import make_identity

P = 128
N, E, G, V, D, H = 150000, 300000, 256, 1000, 128, 256
CORES = 8
GPC = G // CORES
GT = 16        # dst-tiles per psum group
MAXCALL = 8    # max 128-slot blocks per dma_gather call
BF = ml_dtypes.bfloat16
F32 = np.float32
PREPARE = False  # prepare_only+trigger races (NaN) — keep direct gathers


def _idx16_layout(idx):
    n = idx.shape[0]
    s = n // 16
    out = np.zeros((P, s), np.int16)
    blk = idx.reshape(s, 16).T.astype(np.int16)
    for g in range(8):
        out[g * 16:(g + 1) * 16, :] = blk
    return out


def _build_stream(counts2d, ngrp, tiles):
    """counts2d [tiles, n_own]. Returns (segs, total_slots).
    segs: per (group, owner): (owner, slot_off, nblk, incidences)
    incidences: (local_blk, tile, a, b) row-ranges inside each block."""
    n_own = counts2d.shape[1]
    segs = []
    off = 0
    for g in range(ngrp):
        tl = list(range(g * GT, min((g + 1) * GT, tiles)))
        for o in range(n_own):
            cnts = [int(counts2d[t, o]) for t in tl]
            seglen = sum(cnts)
            nblk = max(1, (seglen + P - 1) // P)
            inc = []
            r = 0
            for t, k in zip(tl, cnts):
                if k == 0:
                    continue
                r0, r1 = r, r + k
                for b in range(r0 // P, (r1 - 1) // P + 1):
                    a = max(r0 - b * P, 0)
                    bb = min(r1 - b * P, P)
                    if bb > a:
                        inc.append((b, t, a, bb))
                r = r1
            segs.append((o, off, nblk, inc))
            off += nblk * P
    return segs, off


def _make_plan(segs, ngrp, inc_base):
    """Per group: calls [(owner, slot_off, nblk, blk0)] and tiles
    [(t, [(inc_id, bglob)])]; inc ids sequential in builder order."""
    per_g = len(segs) // ngrp
    plan = []
    inc_id = inc_base
    for g in range(ngrp):
        gsegs = segs[g * per_g:(g + 1) * per_g]
        calls = []
        for (o, off, nblk, inc) in gsegs:
            done = 0
            while done < nblk:
                nb = min(MAXCALL, nblk - done)
                calls.append((o, off + done * P, nb, off // P + done))
                done += nb
        tset = sorted({t for (_, _, _, inc) in gsegs for (_, t, _, _) in inc})
        tiles = []
        for t in tset:
            lst = []
            for (o, off, nblk, inc) in gsegs:
                for (b, tt, a, bb) in inc:
                    if tt == t:
                        lst.append((inc_id, off // P + b, a, bb))
                        inc_id += 1
            tiles.append((t, lst))
        plan.append((calls, tiles))
    return plan, inc_id


def _preprocess(node_ids, edge_index, batch, emb_np):
    node_ids = np.asarray(node_ids).astype(np.int64)
    src_g = np.asarray(edge_index)[0].astype(np.int64)
    dst_g = np.asarray(edge_index)[1].astype(np.int64)
    batch = np.asarray(batch).astype(np.int64)

    deg = 1.0 + np.bincount(dst_g, minlength=N).astype(np.float64)
    dinv = (1.0 / np.sqrt(deg)).astype(F32)
    selfnorm = (dinv * dinv).astype(F32)

    gcnt = np.bincount(batch, minlength=G)
    goff = np.zeros(G + 1, np.int64)
    goff[1:] = np.cumsum(gcnt)
    node_start = np.array([int(goff[c * GPC]) for c in range(CORES + 1)])
    n_c = node_start[1:] - node_start[:-1]
    TILES = int(np.ceil(n_c.max() / P))
    SLAB = TILES * P
    assert SLAB <= 32767, SLAB
    NGRP = (TILES + GT - 1) // GT

    e_oc = np.searchsorted(node_start[1:], dst_g, side="right")
    e_os = np.searchsorted(node_start[1:], src_g, side="right")

    # chunked AllGather split point (collective only; streams stay per-owner)
    TC0 = (TILES // 2 + GT - 1) // GT * GT
    R0, R1 = TC0 * P, SLAB - TC0 * P

    cores = []
    for c in range(CORES):
        m = e_oc == c
        cores.append(dict(src=src_g[m], dstl=dst_g[m] - node_start[c],
                          own=e_os[m], n=int(n_c[c]), nlo=int(node_start[c])))

    # uniform per-(tile, owner) counts
    K1 = np.zeros((TILES, CORES), np.int64)
    K0 = np.zeros((TILES, 1), np.int64)
    for c in range(CORES):
        t_of = cores[c]["dstl"] // P
        cnt = np.zeros((TILES, CORES), np.int64)
        np.add.at(cnt, (t_of, cores[c]["own"]), 1)
        np.maximum(K1, cnt, out=K1)
        c0 = np.bincount(t_of, minlength=TILES).reshape(-1, 1)
        np.maximum(K0, c0, out=K0)

    segs1, S1 = _build_stream(K1, NGRP, TILES)
    segs0, S0 = _build_stream(K0, NGRP, TILES)
    NB0, NB1 = S0 // P, S1 // P
    plan0, ninc0 = _make_plan(segs0, NGRP, 0)
    plan1, ninc1 = _make_plan(segs1, NGRP, ninc0)

    # graph cover ranges (uniform across cores)
    glo_all = np.stack([goff[c * GPC:(c + 1) * GPC] - node_start[c]
                        for c in range(CORES)])   # [CORES, GPC]
    ghi_all = np.stack([goff[c * GPC + 1:(c + 1) * GPC + 1] - node_start[c]
                        for c in range(CORES)])
    lo_fix = glo_all.min(0)
    hi_fix = ghi_all.max(0)
    MAXCOV = int((hi_fix - lo_fix).max())
    MAXCOV = (MAXCOV + 31) // 32 * 32
    cov_len = np.minimum(MAXCOV, SLAB - lo_fix)

    per_core = []
    for c in range(CORES):
        cd = cores[c]
        t_of = cd["dstl"] // P
        idx0 = np.zeros(S0, np.int16)
        idx1 = np.zeros(S1, np.int16)
        scol0 = np.zeros(S0, np.int64)
        scoef0 = np.zeros(S0, F32)
        scol1 = np.zeros(S1, np.int64)
        scoef1 = np.zeros(S1, F32)

        # --- L123 stream fill ---
        order = np.lexsort((cd["dstl"], t_of, cd["own"], t_of // GT))
        srcs = cd["src"][order]
        dstl = cd["dstl"][order]
        owns = cd["own"][order]
        tof = t_of[order]
        coefs = dinv[srcs] * dinv[dstl + cd["nlo"]]
        ptr = 0
        for (o, off, nblk, inc) in segs1:
            starts = {}
            for (b, t, a, bb) in inc:
                if t not in starts:
                    starts[t] = off + b * P + a
            for t in sorted(starts):
                p = starts[t]
                while ptr < len(srcs) and tof[ptr] == t and owns[ptr] == o:
                    idx1[p] = srcs[ptr] - node_start[o]
                    scol1[p] = dstl[ptr] % P
                    scoef1[p] = coefs[ptr]
                    p += 1
                    ptr += 1
        assert ptr == len(srcs), (c, ptr, len(srcs))

        # --- L0 stream fill (edges only; table = T0 by vocab id) ---
        order0 = np.lexsort((cd["dstl"],))
        s0 = cd["src"][order0]
        d0 = cd["dstl"][order0]
        c0 = dinv[s0] * dinv[d0 + cd["nlo"]]
        ptr = 0
        for (o, off, nblk, inc) in segs0:
            starts = {}
            for (b, t, a, bb) in inc:
                if t not in starts:
                    starts[t] = off + b * P + a
            for t in sorted(starts):
                p = starts[t]
                while ptr < len(s0) and d0[ptr] // P == t:
                    idx0[p] = node_ids[s0[ptr]]
                    scol0[p] = d0[ptr] % P
                    scoef0[p] = c0[ptr]
                    p += 1
                    ptr += 1
        assert ptr == len(s0)

        # self-loop term via host-precomputed scaled embedding rows
        xs = np.zeros((SLAB, D), F32)
        ids_own = node_ids[cd["nlo"]:cd["nlo"] + cd["n"]]
        xs[:cd["n"], :] = emb_np[ids_own, :] * \
            selfnorm[cd["nlo"]:cd["nlo"] + cd["n"], None]
        xselfT = np.ascontiguousarray(xs.T.astype(BF))

        seg = np.zeros((ninc1, P, P), F32)
        for plan, scol, scoef in ((plan0, scol0, scoef0),
                                  (plan1, scol1, scoef1)):
            for (calls, tiles) in plan:
                for (t, lst) in tiles:
                    for (inc_id, bglob, a, bb) in lst:
                        rows = np.arange(a, bb)
                        slots = bglob * P + rows
                        seg[inc_id, rows, scol[slots]] = scoef[slots]

        selfn = np.zeros((P, TILES), F32)
        ar = np.arange(cd["n"])
        selfn[ar % P, ar // P] = selfnorm[cd["nlo"]:cd["nlo"] + cd["n"]]

        # graph masks [GPC, P, MAXCOV] bf16 (replicated across partitions)
        mask = np.zeros((GPC, MAXCOV), F32)
        cinv = np.zeros(GPC, F32)
        for j in range(GPC):
            lo = int(glo_all[c, j]) - int(lo_fix[j])
            hi = int(ghi_all[c, j]) - int(lo_fix[j])
            mask[j, lo:hi] = 1.0
            cnt = int(ghi_all[c, j] - glo_all[c, j])
            cinv[j] = 1.0 / max(cnt, 1)
        maskb = np.broadcast_to(mask[:, None, :], (GPC, P, MAXCOV))

        per_core.append(dict(
            idx16=_idx16_layout(np.concatenate([idx0, idx1]).astype(np.int16)),
            seg=np.ascontiguousarray(seg.astype(BF).transpose(1, 0, 2)),
            xselfT=xselfT,
            selfn=selfn,
            mask=np.ascontiguousarray(maskb.astype(BF)),
            cinv=np.broadcast_to(cinv, (P, GPC)).copy(),
        ))

    meta = dict(TILES=TILES, SLAB=SLAB, NGRP=NGRP, NB0=NB0, NB1=NB1,
                S0=S0, S1=S1, segs0=segs0, segs1=segs1,
                plan0=plan0, plan1=plan1, NINC=ninc1,
                lo_fix=lo_fix, cov_len=cov_len, MAXCOV=MAXCOV,
                TC0=TC0, R0=R0, R1=R1)
    return per_core, meta


def _build(meta):
    TILES, SLAB, NGRP = meta["TILES"], meta["SLAB"], meta["NGRP"]
    NB0, NB1 = meta["NB0"], meta["NB1"]
    MAXCOV = meta["MAXCOV"]
    f32, bf16, i16 = mybir.dt.float32, mybir.dt.bfloat16, mybir.dt.int16
    AF = mybir.ActivationFunctionType
    nc = bacc.Bacc("TRN2", target_bir_lowering=False, debug=False,
                   num_devices=CORES)

    SCOLS = (meta["S0"] + meta["S1"]) // 16
    idx_in = nc.dram_tensor("idx16", [P, SCOLS], i16, kind="ExternalInput")
    seg_in = nc.dram_tensor("seg", [P, meta["NINC"], P], bf16,
                            kind="ExternalInput")
    selfn_in = nc.dram_tensor("selfn", [P, TILES], f32, kind="ExternalInput")
    xselfT_in = nc.dram_tensor("xselfT", [P, SLAB], bf16,
                               kind="ExternalInput")
    mask_in = nc.dram_tensor("mask", [GPC, P, MAXCOV], bf16,
                             kind="ExternalInput")
    cinv_in = nc.dram_tensor("cinv", [P, GPC], f32, kind="ExternalInput")
    embT_in = nc.dram_tensor("embT", [P, 1024], bf16, kind="ExternalInput")
    w0_in = nc.dram_tensor("w0", [P, H], bf16, kind="ExternalInput")
    wl_in = nc.dram_tensor("wl", [P, 3, 2, H], bf16, kind="ExternalInput")
    b_in = nc.dram_tensor("bias", [P, 4, H], f32, kind="ExternalInput")
    cw0_in = nc.dram_tensor("cw0", [P, 4, H], bf16, kind="ExternalInput")
    cb0_in = nc.dram_tensor("cb0", [GPC, H], f32, kind="ExternalInput")
    cw1_in = nc.dram_tensor("cw1", [P, 2, 2], bf16, kind="ExternalInput")
    cb1_in = nc.dram_tensor("cb1", [GPC, 2], f32, kind="ExternalInput")
    out = nc.dram_tensor("out", [GPC, 2], f32, kind="ExternalOutput")

    T0_d = nc.dram_tensor("T0d", [1024, H], bf16)

    with tile.TileContext(nc) as tc:
        with (
            tc.tile_pool(name="const", bufs=1) as cpool,
            tc.tile_pool(name="gat", bufs=3) as gpool,
            tc.tile_pool(name="segp", bufs=3) as segp,
            tc.tile_pool(name="selfp", bufs=4) as selfp,
            tc.tile_pool(name="work", bufs=4) as work,
            tc.tile_pool(name="hmp", bufs=2) as hmp,
            tc.tile_pool(name="ps", bufs=4, space="PSUM") as ps,
            tc.tile_pool(name="pstr", bufs=2, space="PSUM") as pstr,
            tc.tile_pool(name="psmm", bufs=2, space="PSUM") as psmm,
            tc.tile_pool(name="dram", bufs=1, space="DRAM") as dpool,
        ):
            idx_t = cpool.tile([P, SCOLS], i16)
            nc.sync.dma_start(out=idx_t[:], in_=idx_in[:, :])
            selfn_t = cpool.tile([P, TILES], f32)
            nc.sync.dma_start(out=selfn_t[:], in_=selfn_in[:, :])
            cinv_t = cpool.tile([P, GPC], f32)
            nc.sync.dma_start(out=cinv_t[:], in_=cinv_in[:, :])
            embT_t = cpool.tile([P, 1024], bf16)
            nc.sync.dma_start(out=embT_t[:], in_=embT_in[:, :])
            w0_t = cpool.tile([P, H], bf16)
            nc.sync.dma_start(out=w0_t[:], in_=w0_in[:, :])
            wl_t = cpool.tile([P, 3, 2, H], bf16)
            nc.sync.dma_start(out=wl_t[:], in_=wl_in[:, :, :, :])
            bias_t = cpool.tile([P, 4, H], f32)
            nc.sync.dma_start(out=bias_t[:], in_=b_in[:, :, :])

            h_T = nc.alloc_sbuf_tensor("hT", [P, 2, SLAB], bf16)
            ident = cpool.tile([P, P], bf16)
            make_identity(nc, ident[:])

            # ---- T0 = emb @ w0 ----
            for vb in range(8):
                pt0 = psmm.tile([P, H], f32, tag="mm")
                nc.tensor.matmul(out=pt0[:], lhsT=embT_t[:, vb * P:(vb + 1) * P],
                                 rhs=w0_t[:], start=True, stop=True)
                t0s = work.tile([P, H], bf16)
                nc.scalar.copy(out=t0s[:], in_=pt0[:])
                nc.sync.dma_start(out=T0_d[vb * P:(vb + 1) * P, :], in_=t0s[:])

            TC0, R0, R1 = meta["TC0"], meta["R0"], meta["R1"]
            # [parity] double-buffered exchange tensors: the AllGathers for
            # layer L+1 fire while spmm(L) still reads layer L's data
            MBs, MFs = [], []
            for par in range(2):
                mb = nc.dram_tensor(f"MBd{par}", [SLAB, H], bf16)
                mf = nc.dram_tensor(f"MFd{par}", [CORES, SLAB, H], bf16,
                                    addr_space="Shared")
                MBs.append(mb)
                MFs.append(mf)

            # one semaphore per SWDGE lane, cycled in Pool-DMA emission order
            # to match tile_sem_assignment's next_sw_dma_idx rotation
            dma_sems = [nc.alloc_semaphore(f"swdge_dma{i}") for i in range(8)]
            prep_ct = [0]

            def gmeta(plan):
                out = []
                for (calls, tiles) in plan:
                    b0 = min(c[3] for c in calls)
                    nb = sum(c[2] for c in calls)
                    out.append((b0, nb))
                return out
            gm0, gm1 = gmeta(meta["plan0"]), gmeta(meta["plan1"])
            NBGMAX = max(nb for (_, nb) in gm0 + gm1)
            PF = 3  # gather group prefetch depth == gpool bufs

            def spmm(layer, hooks=None):
                plan = meta["plan0"] if layer == 0 else meta["plan1"]
                gmv = gm0 if layer == 0 else gm1
                base_slot = 0 if layer == 0 else meta["S0"]
                NG = len(plan)
                gt_map = {}

                def prep(g):
                    calls, _ = plan[g]
                    b0, _ = gmv[g]
                    gt_ = gpool.tile([P, NBGMAX, H], bf16, tag="gtg")
                    gt_map[g] = (gt_, b0)
                    for (o2, soff, nb, blk0) in calls:
                        slot0 = base_slot + soff
                        if layer == 0:
                            tab = T0_d[0:1024, :]
                        else:
                            tab = MFs[layer % 2][o2, :, :]
                        sem = dma_sems[prep_ct[0] % 8] if PREPARE else None
                        prep_ct[0] += 1
                        nc.gpsimd.dma_gather(
                            out_ap=gt_[:, blk0 - b0:blk0 - b0 + nb, :],
                            in_ap=tab,
                            idxs_ap=idx_t[:, slot0 // 16:
                                          (slot0 + nb * P) // 16],
                            num_idxs=nb * P, num_idxs_reg=nb * P,
                            elem_size=H, queue_num=0,
                            prepare_only=PREPARE, sem=sem)
                    if PREPARE:
                        nc.gpsimd.trigger_dma(count=None)

                for g in range(min(PF, NG)):
                    prep(g)
                for g, (calls, tiles) in enumerate(plan):
                    gt_, b0 = gt_map.pop(g)
                    tmap = dict(tiles)
                    for t in range(g * GT, min((g + 1) * GT, TILES)):
                        lst = tmap.get(t, [])
                        nL = len(lst)
                        pt = ps.tile([P, H], f32)
                        if layer > 0:
                            ssrc = selfp.tile([P, H], bf16, tag="ss")
                            nc.sync.dma_start(
                                out=ssrc[:],
                                in_=MBs[layer % 2][t * P:(t + 1) * P, :])
                            nc.scalar.activation(
                                out=pt[:], in_=ssrc[:], func=AF.Copy,
                                scale=selfn_t[:, t:t + 1])
                        else:
                            xs = selfp.tile([P, P], bf16, tag="xs")
                            nc.sync.dma_start(
                                out=xs[:],
                                in_=xselfT_in[:, t * P:(t + 1) * P])
                            nc.tensor.matmul(
                                out=pt[:], lhsT=xs[:], rhs=w0_t[:],
                                start=True, stop=(nL == 0))
                        if nL:
                            i0 = lst[0][0]
                            st = segp.tile([P, nL, P], bf16, tag="st")
                            nc.scalar.dma_start(out=st[:, :, :],
                                                in_=seg_in[:, i0:i0 + nL, :])
                        for n_i, (inc_id, bglob, a, bb) in enumerate(lst):
                            nc.tensor.matmul(
                                out=pt[:], lhsT=st[:, n_i, :],
                                rhs=gt_[:, bglob - b0, :],
                                start=False,
                                stop=(n_i == nL - 1),
                                skip_group_check=(layer > 0))
                        tmp = work.tile([P, H], f32)
                        nc.vector.tensor_add(out=tmp[:], in0=pt[:],
                                             in1=bias_t[:, layer, :])
                        tmpb = work.tile([P, H], bf16)
                        nc.scalar.activation(out=tmpb[:], in_=tmp[:],
                                             func=AF.Relu)
                        for fh in range(2):
                            ptr_ = pstr.tile([P, P], bf16)
                            nc.tensor.transpose(
                                out=ptr_[:], in_=tmpb[:, fh * P:(fh + 1) * P],
                                identity=ident[:])
                            nc.vector.tensor_copy(
                                out=h_T[:, fh, t * P:(t + 1) * P], in_=ptr_[:])
                    if g + PF < NG:
                        prep(g + PF)
                    if hooks and g in hooks:
                        hooks[g]()

            def emit_m(layer, chunk):
                # M = h @ W for one tile chunk -> MB, then AllGather chunk
                # (strided output: each core's rows land inside its slab)
                t0, t1 = (0, TC0) if chunk == 0 else (TC0, TILES)
                MB = MBs[layer % 2]
                MF = MFs[layer % 2]
                for t in range(t0, t1):
                    pm = psmm.tile([P, H], f32, tag="mm")
                    for fh in range(2):
                        nc.tensor.matmul(
                            out=pm[:],
                            lhsT=h_T[:, fh, t * P:(t + 1) * P],
                            rhs=wl_t[:, layer - 1, fh, :],
                            start=(fh == 0), stop=(fh == 1))
                    mt = work.tile([P, H], bf16)
                    nc.scalar.copy(out=mt[:], in_=pm[:])
                    nc.sync.dma_start(
                        out=MB[t * P:(t + 1) * P, :], in_=mt[:])
                if chunk == 1:
                    nc.gpsimd.collective_compute(
                        "AllGather", mybir.AluOpType.bypass,
                        replica_groups=[list(range(CORES))],
                        ins=[MB[:].opt()],
                        outs=[MF[:, :, :].opt()])

            # ---- pooling state (masked mean/max on h_T, emitted per-graph
            # from spmm(3) hooks as soon as covering tiles are done) ----
            pooled = []
            for fh in range(2):
                mean_t = cpool.tile([P, GPC], f32, tag=f"mean{fh}")
                max_t = cpool.tile([P, GPC], f32, tag=f"max{fh}")
                nc.vector.memset(mean_t[:], 0.0)
                nc.vector.memset(max_t[:], 0.0)
                pooled.append((mean_t, max_t))
            lo_fix, cov_len = meta["lo_fix"], meta["cov_len"]

            def pool_graph(j):
                ln = int(cov_len[j])
                lo = int(lo_fix[j])
                mka = hmp.tile([P, MAXCOV], bf16, tag="mask")
                mkb = hmp.tile([P, MAXCOV], bf16, tag="mask")
                mks = [mka, mkb]
                nc.sync.dma_start(out=mks[0][:, 0:ln],
                                  in_=mask_in[j, :, 0:ln])
                nc.vector.tensor_copy(out=mks[1][:, 0:ln],
                                      in_=mks[0][:, 0:ln])
                for fh in range(2):
                    mk = mks[fh]
                    nc.vector.tensor_mul(out=mk[:, 0:ln],
                                         in0=h_T[:, fh, lo:lo + ln],
                                         in1=mk[:, 0:ln])
                    nc.vector.tensor_reduce(
                        out=pooled[fh][0][:, j:j + 1], in_=mk[:, 0:ln],
                        axis=mybir.AxisListType.X, op=mybir.AluOpType.add)
                    nc.vector.tensor_reduce(
                        out=pooled[fh][1][:, j:j + 1], in_=mk[:, 0:ln],
                        axis=mybir.AxisListType.X, op=mybir.AluOpType.max)

            # graphs ready after each group of spmm(3)
            ready_at = [[] for _ in range(NGRP)]
            for j in range(GPC):
                need = int(lo_fix[j]) + int(cov_len[j])
                g = max(0, min(NGRP - 1, (need + GT * P - 1) // (GT * P) - 1))
                ready_at[g].append(j)

            def pool_hook(g):
                for j in ready_at[g]:
                    pool_graph(j)

            GSPLIT = TC0 // GT - 1   # group after which chunk-0 h is ready
            for layer in (0, 1, 2, 3):
                if layer < 3:
                    hooks = {GSPLIT: (lambda l: lambda: emit_m(l, 0))(layer + 1),
                             NGRP - 1: (lambda l: lambda: emit_m(l, 1))(layer + 1)}
                else:
                    hooks = {g: (lambda gg: lambda: pool_hook(gg))(g)
                             for g in range(NGRP)}
                spmm(layer, hooks=hooks)
            # scale means by 1/cnt, cast to bf16 lhsT chunks
            chunks = []
            for fh in range(2):
                mean_t, max_t = pooled[fh]
                nc.vector.tensor_mul(out=mean_t[:], in0=mean_t[:],
                                     in1=cinv_t[:])
            for (kind, fh) in ((0, 0), (0, 1), (1, 0), (1, 1)):
                src = pooled[fh][kind]
                cb = work.tile([P, GPC], bf16, tag=f"ch{kind}{fh}")
                nc.vector.tensor_copy(out=cb[:], in_=src[:])
                chunks.append(cb)

            # ---- classifier MLP ----
            cw0_t = cpool.tile([P, 4, H], bf16)
            nc.sync.dma_start(out=cw0_t[:], in_=cw0_in[:, :, :])
            cb0_t = cpool.tile([GPC, H], f32)
            nc.sync.dma_start(out=cb0_t[:], in_=cb0_in[:, :])
            cw1_t = cpool.tile([P, 2, 2], bf16)
            nc.sync.dma_start(out=cw1_t[:], in_=cw1_in[:, :, :])
            cb1_t = cpool.tile([GPC, 2], f32)
            nc.sync.dma_start(out=cb1_t[:], in_=cb1_in[:, :])

            ph_full = psmm.tile([P, H], f32, tag="mm")
            ph = ph_full[0:GPC, :]
            for k in range(4):
                nc.tensor.matmul(out=ph[:], lhsT=chunks[k][:],
                                 rhs=cw0_t[:, k, :],
                                 start=(k == 0), stop=(k == 3))
            hc1 = work.tile([GPC, H], f32, tag="hc1")
            nc.vector.tensor_add(out=hc1[:], in0=ph[:], in1=cb0_t[:])
            hcb = work.tile([GPC, H], bf16, tag="hcb")
            nc.scalar.activation(out=hcb[:], in_=hc1[:], func=AF.Relu)
            hTt = []
            for k in range(2):
                ptr_ = pstr.tile([P, P], bf16)
                nc.tensor.transpose(out=ptr_[0:P, 0:GPC],
                                    in_=hcb[:, k * P:(k + 1) * P],
                                    identity=ident[0:GPC, 0:GPC])
                ht = work.tile([P, GPC], bf16, tag=f"hTt{k}")
                nc.vector.tensor_copy(out=ht[:], in_=ptr_[0:P, 0:GPC])
                hTt.append(ht)
            pl_full = psmm.tile([P, H], f32, tag="mm")
            pl = pl_full[0:GPC, 0:2]
            for k in range(2):
                nc.tensor.matmul(out=pl[:], lhsT=hTt[k][:],
                                 rhs=cw1_t[:, k, :],
                                 start=(k == 0), stop=(k == 1))
            lg = work.tile([GPC, 2], f32, tag="lg")
            nc.vector.tensor_add(out=lg[:], in0=pl[:], in1=cb1_t[:])
            nc.sync.dma_start(out=out[:, :], in_=lg[:])
    nc.finalize()
    return nc


def kernel(node_ids, edge_index, batch, emb, w0, b0, w1, b1, w2, b2, w3, b3,
           cw0, cb0, cw1, cb1):
    per_core, meta = _preprocess(node_ids, edge_index, batch,
                                 np.asarray(emb, F32))
    nc = _build(meta)

    embT = np.zeros((P, 1024), F32)
    embT[:, :V] = np.asarray(emb, F32).T
    wlk = np.transpose(np.stack([np.asarray(w, F32).reshape(2, P, H)
                    for w in (w1, w2, w3)]), (2, 0, 1, 3)).copy()
    biases = np.stack([np.broadcast_to(np.asarray(b, F32), (P, H))
                       for b in (b0, b1, b2, b3)], axis=1).copy()
    ins = []
    for c in range(CORES):
        pc = per_core[c]
        ins.append(dict(
            idx16=pc["idx16"], seg=pc["seg"], selfn=pc["selfn"],
            xselfT=pc["xselfT"], mask=pc["mask"], cinv=pc["cinv"],
            embT=embT.astype(BF), w0=np.asarray(w0, F32).astype(BF),
            wl=wlk.astype(BF), bias=biases.astype(F32),
            cw0=np.transpose(np.asarray(cw0, F32).reshape(4, P, H), (1, 0, 2)).astype(BF),
            cb0=np.broadcast_to(np.asarray(cb0, F32), (GPC, H)).copy(),
            cw1=np.transpose(np.asarray(cw1, F32).reshape(2, P, 2), (1, 0, 2)).astype(BF),
            cb1=np.broadcast_to(np.asarray(cb1, F32), (GPC, 2)).copy(),
        ))
    trace = False
    try:  # register NTFF hook so exec_time_ns is measurable (best effort)
        import sys, types
        import antenv
        if "antenv.axon_hooks" not in sys.modules:
            hooks = types.ModuleType("antenv.axon_hooks")
            hooks._h = None
            hooks.set_axon_ntff_profile_hook = lambda h: setattr(hooks, "_h", h)
            hooks.get_axon_ntff_profile_hook = lambda: hooks._h
            sys.modules["antenv.axon_hooks"] = hooks
            antenv.axon_hooks = hooks
            from trn_agent_boot.trn_boot import _ntff_profile_via_ctypes
            hk = _ntff_profile_via_ctypes("/opt/axon/libaxon_pjrt.so")
            if hk is not None:
                hooks.set_axon_ntff_profile_hook(hk)
                trace = True
        else:
            trace = True
    except Exception:
        trace = False
    res = run_bass_kernel_spmd(nc, ins, core_ids=list(range(CORES)),
                               trace=trace)
    logits = np.concatenate([res.results[c]["out"] for c in range(CORES)], 0)
    globals()["last_exec_ns"] = res.exec_time_ns
    globals()["last_res"] = res
    return logits.astype(np.float32)



# revision 76
# speedup vs baseline: 2.1330x; 1.0010x over previous
"""GCN (EnhancedTaintFlowGNN) on 8 Trainium2 NeuronCores.

Sharding: 32 graphs/core (batch sorted -> contiguous node ranges). Per GCN
layer: M = h @ W locally (PE), AllGather M, pull-side SpMM via dma_gather
from 8 owner-aligned int16 tables + PE segment-matmuls (norm folded into the
segment coefficients). Self-loops preloaded into PSUM by ACT scaled by
dinv^2. Layer 0 gathers rows of T0 = emb @ w0 (vocab table, replicated on
every core) so no exchange is needed. Pooling: masked DVE mean/max on
transposed features; classifier MLP on PE.
"""
import numpy as np
import ml_dtypes

import concourse.bass as bass
import concourse.bacc as bacc
import concourse.tile as tile
from concourse import mybir
from concourse.bass_utils import run_bass_kernel_spmd
from concourse.masks import make_identity

P = 128
N, E, G, V, D, H = 150000, 300000, 256, 1000, 128, 256
CORES = 8
GPC = G // CORES
GT = 16        # dst-tiles per psum group
MAXCALL = 8    # max 128-slot blocks per dma_gather call
BF = ml_dtypes.bfloat16
F32 = np.float32
PREPARE = False  # prepare_only+trigger races (NaN) — keep direct gathers


def _idx16_layout(idx):
    n = idx.shape[0]
    s = n // 16
    out = np.zeros((P, s), np.int16)
    blk = idx.reshape(s, 16).T.astype(np.int16)
    for g in range(8):
        out[g * 16:(g + 1) * 16, :] = blk
    return out


def _build_stream(counts2d, ngrp, tiles):
    """counts2d [tiles, n_own]. Returns (segs, total_slots).
    segs: per (group, owner): (owner, slot_off, nblk, incidences)
    incidences: (local_blk, tile, a, b) row-ranges inside each block."""
    n_own = counts2d.shape[1]
    segs = []
    off = 0
    for g in range(ngrp):
        tl = list(range(g * GT, min((g + 1) * GT, tiles)))
        for o in range(n_own):
            cnts = [int(counts2d[t, o]) for t in tl]
            seglen = sum(cnts)
            nblk = max(1, (seglen + P - 1) // P)
            inc = []
            r = 0
            for t, k in zip(tl, cnts):
                if k == 0:
                    continue
                r0, r1 = r, r + k
                for b in range(r0 // P, (r1 - 1) // P + 1):
                    a = max(r0 - b * P, 0)
                    bb = min(r1 - b * P, P)
                    if bb > a:
                        inc.append((b, t, a, bb))
                r = r1
            segs.append((o, off, nblk, inc))
            off += nblk * P
    return segs, off


def _make_plan(segs, ngrp, inc_base):
    """Per group: calls [(owner, slot_off, nblk, blk0)] and tiles
    [(t, [(inc_id, bglob)])]; inc ids sequential in builder order."""
    per_g = len(segs) // ngrp
    plan = []
    inc_id = inc_base
    for g in range(ngrp):
        gsegs = segs[g * per_g:(g + 1) * per_g]
        calls = []
        for (o, off, nblk, inc) in gsegs:
            done = 0
            while done < nblk:
                nb = min(MAXCALL, nblk - done)
                calls.append((o, off + done * P, nb, off // P + done))
                done += nb
        tset = sorted({t for (_, _, _, inc) in gsegs for (_, t, _, _) in inc})
        tiles = []
        for t in tset:
            lst = []
            for (o, off, nblk, inc) in gsegs:
                for (b, tt, a, bb) in inc:
                    if tt == t:
                        lst.append((inc_id, off // P + b, a, bb))
                        inc_id += 1
            tiles.append((t, lst))
        plan.append((calls, tiles))
    return plan, inc_id


def _preprocess(node_ids, edge_index, batch, emb_np):
    node_ids = np.asarray(node_ids).astype(np.int64)
    src_g = np.asarray(edge_index)[0].astype(np.int64)
    dst_g = np.asarray(edge_index)[1].astype(np.int64)
    batch = np.asarray(batch).astype(np.int64)

    deg = 1.0 + np.bincount(dst_g, minlength=N).astype(np.float64)
    dinv = (1.0 / np.sqrt(deg)).astype(F32)
    selfnorm = (dinv * dinv).astype(F32)

    gcnt = np.bincount(batch, minlength=G)
    goff = np.zeros(G + 1, np.int64)
    goff[1:] = np.cumsum(gcnt)
    node_start = np.array([int(goff[c * GPC]) for c in range(CORES + 1)])
    n_c = node_start[1:] - node_start[:-1]
    TILES = int(np.ceil(n_c.max() / P))
    SLAB = TILES * P
    assert SLAB <= 32767, SLAB
    NGRP = (TILES + GT - 1) // GT

    e_oc = np.searchsorted(node_start[1:], dst_g, side="right")
    e_os = np.searchsorted(node_start[1:], src_g, side="right")

    # chunked AllGather split point (collective only; streams stay per-owner)
    TC0 = (TILES // 2 + GT - 1) // GT * GT
    R0, R1 = TC0 * P, SLAB - TC0 * P

    cores = []
    for c in range(CORES):
        m = e_oc == c
        cores.append(dict(src=src_g[m], dstl=dst_g[m] - node_start[c],
                          own=e_os[m], n=int(n_c[c]), nlo=int(node_start[c])))

    # uniform per-(tile, owner) counts
    K1 = np.zeros((TILES, CORES), np.int64)
    K0 = np.zeros((TILES, 1), np.int64)
    for c in range(CORES):
        t_of = cores[c]["dstl"] // P
        cnt = np.zeros((TILES, CORES), np.int64)
        np.add.at(cnt, (t_of, cores[c]["own"]), 1)
        np.maximum(K1, cnt, out=K1)
        c0 = np.bincount(t_of, minlength=TILES).reshape(-1, 1)
        np.maximum(K0, c0, out=K0)

    segs1, S1 = _build_stream(K1, NGRP, TILES)
    segs0, S0 = _build_stream(K0, NGRP, TILES)
    NB0, NB1 = S0 // P, S1 // P
    plan0, ninc0 = _make_plan(segs0, NGRP, 0)
    plan1, ninc1 = _make_plan(segs1, NGRP, ninc0)

    # graph cover ranges (uniform across cores)
    glo_all = np.stack([goff[c * GPC:(c + 1) * GPC] - node_start[c]
                        for c in range(CORES)])   # [CORES, GPC]
    ghi_all = np.stack([goff[c * GPC + 1:(c + 1) * GPC + 1] - node_start[c]
                        for c in range(CORES)])
    lo_fix = glo_all.min(0)
    hi_fix = ghi_all.max(0)
    MAXCOV = int((hi_fix - lo_fix).max())
    MAXCOV = (MAXCOV + 31) // 32 * 32
    cov_len = np.minimum(MAXCOV, SLAB - lo_fix)

    per_core = []
    for c in range(CORES):
        cd = cores[c]
        t_of = cd["dstl"] // P
        idx0 = np.zeros(S0, np.int16)
        idx1 = np.zeros(S1, np.int16)
        scol0 = np.zeros(S0, np.int64)
        scoef0 = np.zeros(S0, F32)
        scol1 = np.zeros(S1, np.int64)
        scoef1 = np.zeros(S1, F32)

        # --- L123 stream fill ---
        order = np.lexsort((cd["dstl"], t_of, cd["own"], t_of // GT))
        srcs = cd["src"][order]
        dstl = cd["dstl"][order]
        owns = cd["own"][order]
        tof = t_of[order]
        coefs = dinv[srcs] * dinv[dstl + cd["nlo"]]
        ptr = 0
        for (o, off, nblk, inc) in segs1:
            starts = {}
            for (b, t, a, bb) in inc:
                if t not in starts:
                    starts[t] = off + b * P + a
            for t in sorted(starts):
                p = starts[t]
                while ptr < len(srcs) and tof[ptr] == t and owns[ptr] == o:
                    idx1[p] = srcs[ptr] - node_start[o]
                    scol1[p] = dstl[ptr] % P
                    scoef1[p] = coefs[ptr]
                    p += 1
                    ptr += 1
        assert ptr == len(srcs), (c, ptr, len(srcs))

        # --- L0 stream fill (edges only; table = T0 by vocab id) ---
        order0 = np.lexsort((cd["dstl"],))
        s0 = cd["src"][order0]
        d0 = cd["dstl"][order0]
        c0 = dinv[s0] * dinv[d0 + cd["nlo"]]
        ptr = 0
        for (o, off, nblk, inc) in segs0:
            starts = {}
            for (b, t, a, bb) in inc:
                if t not in starts:
                    starts[t] = off + b * P + a
            for t in sorted(starts):
                p = starts[t]
                while ptr < len(s0) and d0[ptr] // P == t:
                    idx0[p] = node_ids[s0[ptr]]
                    scol0[p] = d0[ptr] % P
                    scoef0[p] = c0[ptr]
                    p += 1
                    ptr += 1
        assert ptr == len(s0)

        # self-loop term via host-precomputed scaled embedding rows
        xs = np.zeros((SLAB, D), F32)
        ids_own = node_ids[cd["nlo"]:cd["nlo"] + cd["n"]]
        xs[:cd["n"], :] = emb_np[ids_own, :] * \
            selfnorm[cd["nlo"]:cd["nlo"] + cd["n"], None]
        xselfT = np.ascontiguousarray(xs.T.astype(BF))

        seg = np.zeros((ninc1, P, P), F32)
        for plan, scol, scoef in ((plan0, scol0, scoef0),
                                  (plan1, scol1, scoef1)):
            for (calls, tiles) in plan:
                for (t, lst) in tiles:
                    for (inc_id, bglob, a, bb) in lst:
                        rows = np.arange(a, bb)
                        slots = bglob * P + rows
                        seg[inc_id, rows, scol[slots]] = scoef[slots]

        selfn = np.zeros((P, TILES), F32)
        ar = np.arange(cd["n"])
        selfn[ar % P, ar // P] = selfnorm[cd["nlo"]:cd["nlo"] + cd["n"]]

        # graph masks [GPC, P, MAXCOV] bf16 (replicated across partitions)
        mask = np.zeros((GPC, MAXCOV), F32)
        cinv = np.zeros(GPC, F32)
        for j in range(GPC):
            lo = int(glo_all[c, j]) - int(lo_fix[j])
            hi = int(ghi_all[c, j]) - int(lo_fix[j])
            mask[j, lo:hi] = 1.0
            cnt = int(ghi_all[c, j] - glo_all[c, j])
            cinv[j] = 1.0 / max(cnt, 1)
        maskb = np.broadcast_to(mask[:, None, :], (GPC, P, MAXCOV))

        per_core.append(dict(
            idx16=_idx16_layout(np.concatenate([idx0, idx1]).astype(np.int16)),
            seg=np.ascontiguousarray(seg.astype(BF).transpose(1, 0, 2)),
            xselfT=xselfT,
            selfn=selfn,
            mask=np.ascontiguousarray(maskb.astype(BF)),
            cinv=np.broadcast_to(cinv, (P, GPC)).copy(),
        ))

    meta = dict(TILES=TILES, SLAB=SLAB, NGRP=NGRP, NB0=NB0, NB1=NB1,
                S0=S0, S1=S1, segs0=segs0, segs1=segs1,
                plan0=plan0, plan1=plan1, NINC=ninc1,
                lo_fix=lo_fix, cov_len=cov_len, MAXCOV=MAXCOV,
                TC0=TC0, R0=R0, R1=R1)
    return per_core, meta


def _build(meta):
    TILES, SLAB, NGRP = meta["TILES"], meta["SLAB"], meta["NGRP"]
    NB0, NB1 = meta["NB0"], meta["NB1"]
    MAXCOV = meta["MAXCOV"]
    f32, bf16, i16 = mybir.dt.float32, mybir.dt.bfloat16, mybir.dt.int16
    AF = mybir.ActivationFunctionType
    nc = bacc.Bacc("TRN2", target_bir_lowering=False, debug=False,
                   num_devices=CORES)

    SCOLS = (meta["S0"] + meta["S1"]) // 16
    idx_in = nc.dram_tensor("idx16", [P, SCOLS], i16, kind="ExternalInput")
    seg_in = nc.dram_tensor("seg", [P, meta["NINC"], P], bf16,
                            kind="ExternalInput")
    selfn_in = nc.dram_tensor("selfn", [P, TILES], f32, kind="ExternalInput")
    xselfT_in = nc.dram_tensor("xselfT", [P, SLAB], bf16,
                               kind="ExternalInput")
    mask_in = nc.dram_tensor("mask", [GPC, P, MAXCOV], bf16,
                             kind="ExternalInput")
    cinv_in = nc.dram_tensor("cinv", [P, GPC], f32, kind="ExternalInput")
    embT_in = nc.dram_tensor("embT", [P, 1024], bf16, kind="ExternalInput")
    w0_in = nc.dram_tensor("w0", [P, H], bf16, kind="ExternalInput")
    wl_in = nc.dram_tensor("wl", [P, 3, 2, H], bf16, kind="ExternalInput")
    b_in = nc.dram_tensor("bias", [P, 4, H], f32, kind="ExternalInput")
    cw0_in = nc.dram_tensor("cw0", [P, 4, H], bf16, kind="ExternalInput")
    cb0_in = nc.dram_tensor("cb0", [GPC, H], f32, kind="ExternalInput")
    cw1_in = nc.dram_tensor("cw1", [P, 2, 2], bf16, kind="ExternalInput")
    cb1_in = nc.dram_tensor("cb1", [GPC, 2], f32, kind="ExternalInput")
    out = nc.dram_tensor("out", [GPC, 2], f32, kind="ExternalOutput")

    T0_d = nc.dram_tensor("T0d", [1024, H], bf16)

    with tile.TileContext(nc) as tc:
        with (
            tc.tile_pool(name="const", bufs=1) as cpool,
            tc.tile_pool(name="gat", bufs=3) as gpool,
            tc.tile_pool(name="segp", bufs=3) as segp,
            tc.tile_pool(name="selfp", bufs=4) as selfp,
            tc.tile_pool(name="work", bufs=4) as work,
            tc.tile_pool(name="hmp", bufs=2) as hmp,
            tc.tile_pool(name="ps", bufs=4, space="PSUM") as ps,
            tc.tile_pool(name="pstr", bufs=2, space="PSUM") as pstr,
            tc.tile_pool(name="psmm", bufs=2, space="PSUM") as psmm,
            tc.tile_pool(name="dram", bufs=1, space="DRAM") as dpool,
        ):
            idx_t = cpool.tile([P, SCOLS], i16)
            nc.sync.dma_start(out=idx_t[:], in_=idx_in[:, :])
            selfn_t = cpool.tile([P, TILES], f32)
            nc.sync.dma_start(out=selfn_t[:], in_=selfn_in[:, :])
            cinv_t = cpool.tile([P, GPC], f32)
            nc.sync.dma_start(out=cinv_t[:], in_=cinv_in[:, :])
            embT_t = cpool.tile([P, 1024], bf16)
            nc.sync.dma_start(out=embT_t[:], in_=embT_in[:, :])
            w0_t = cpool.tile([P, H], bf16)
            nc.sync.dma_start(out=w0_t[:], in_=w0_in[:, :])
            wl_t = cpool.tile([P, 3, 2, H], bf16)
            nc.sync.dma_start(out=wl_t[:], in_=wl_in[:, :, :, :])
            bias_t = cpool.tile([P, 4, H], f32)
            nc.sync.dma_start(out=bias_t[:], in_=b_in[:, :, :])
            cw0_t = cpool.tile([P, 4, H], bf16)
            nc.sync.dma_start(out=cw0_t[:], in_=cw0_in[:, :, :])
            cb0_t = cpool.tile([GPC, H], f32)
            nc.sync.dma_start(out=cb0_t[:], in_=cb0_in[:, :])
            cw1_t = cpool.tile([P, 2, 2], bf16)
            nc.sync.dma_start(out=cw1_t[:], in_=cw1_in[:, :, :])
            cb1_t = cpool.tile([GPC, 2], f32)
            nc.sync.dma_start(out=cb1_t[:], in_=cb1_in[:, :])

            h_T = nc.alloc_sbuf_tensor("hT", [P, 2, SLAB], bf16)
            ident = cpool.tile([P, P], bf16)
            make_identity(nc, ident[:])

            # ---- T0 = emb @ w0 ----
            for vb in range(8):
                pt0 = psmm.tile([P, H], f32, tag="mm")
                nc.tensor.matmul(out=pt0[:], lhsT=embT_t[:, vb * P:(vb + 1) * P],
                                 rhs=w0_t[:], start=True, stop=True)
                t0s = work.tile([P, H], bf16)
                nc.scalar.copy(out=t0s[:], in_=pt0[:])
                nc.sync.dma_start(out=T0_d[vb * P:(vb + 1) * P, :], in_=t0s[:])

            TC0, R0, R1 = meta["TC0"], meta["R0"], meta["R1"]
            # [parity] double-buffered exchange tensors: the AllGathers for
            # layer L+1 fire while spmm(L) still reads layer L's data
            MBs, MFs = [], []
            for par in range(2):
                mb = nc.dram_tensor(f"MBd{par}", [SLAB, H], bf16)
                mf = nc.dram_tensor(f"MFd{par}", [CORES, SLAB, H], bf16,
                                    addr_space="Shared")
                MBs.append(mb)
                MFs.append(mf)

            # one semaphore per SWDGE lane, cycled in Pool-DMA emission order
            # to match tile_sem_assignment's next_sw_dma_idx rotation
            dma_sems = [nc.alloc_semaphore(f"swdge_dma{i}") for i in range(8)]
            prep_ct = [0]

            def gmeta(plan):
                out = []
                for (calls, tiles) in plan:
                    b0 = min(c[3] for c in calls)
                    nb = sum(c[2] for c in calls)
                    out.append((b0, nb))
                return out
            gm0, gm1 = gmeta(meta["plan0"]), gmeta(meta["plan1"])
            NBGMAX = max(nb for (_, nb) in gm0 + gm1)
            PF = 3  # gather group prefetch depth == gpool bufs

            def spmm(layer, hooks=None):
                plan = meta["plan0"] if layer == 0 else meta["plan1"]
                gmv = gm0 if layer == 0 else gm1
                base_slot = 0 if layer == 0 else meta["S0"]
                NG = len(plan)
                gt_map = {}

                def prep(g):
                    calls, _ = plan[g]
                    b0, _ = gmv[g]
                    gt_ = gpool.tile([P, NBGMAX, H], bf16, tag="gtg")
                    gt_map[g] = (gt_, b0)
                    for (o2, soff, nb, blk0) in calls:
                        slot0 = base_slot + soff
                        if layer == 0:
                            tab = T0_d[0:1024, :]
                        else:
                            tab = MFs[layer % 2][o2, :, :]
                        sem = dma_sems[prep_ct[0] % 8] if PREPARE else None
                        prep_ct[0] += 1
                        nc.gpsimd.dma_gather(
                            out_ap=gt_[:, blk0 - b0:blk0 - b0 + nb, :],
                            in_ap=tab,
                            idxs_ap=idx_t[:, slot0 // 16:
                                          (slot0 + nb * P) // 16],
                            num_idxs=nb * P, num_idxs_reg=nb * P,
                            elem_size=H, queue_num=0,
                            single_packet=False,
                            prepare_only=PREPARE, sem=sem)
                    if PREPARE:
                        nc.gpsimd.trigger_dma(count=None)

                for g in range(min(PF, NG)):
                    prep(g)
                for g, (calls, tiles) in enumerate(plan):
                    gt_, b0 = gt_map.pop(g)
                    tmap = dict(tiles)
                    for t in range(g * GT, min((g + 1) * GT, TILES)):
                        lst = tmap.get(t, [])
                        nL = len(lst)
                        pt = ps.tile([P, H], f32)
                        if layer > 0:
                            ssrc = selfp.tile([P, H], bf16, tag="ss")
                            nc.sync.dma_start(
                                out=ssrc[:],
                                in_=MBs[layer % 2][t * P:(t + 1) * P, :])
                            nc.scalar.activation(
                                out=pt[:], in_=ssrc[:], func=AF.Copy,
                                scale=selfn_t[:, t:t + 1])
                        else:
                            xs = selfp.tile([P, P], bf16, tag="xs")
                            nc.sync.dma_start(
                                out=xs[:],
                                in_=xselfT_in[:, t * P:(t + 1) * P])
                            nc.tensor.matmul(
                                out=pt[:], lhsT=xs[:], rhs=w0_t[:],
                                start=True, stop=(nL == 0))
                        if nL:
                            i0 = lst[0][0]
                            st = segp.tile([P, nL, P], bf16, tag="st")
                            nc.scalar.dma_start(out=st[:, :, :],
                                                in_=seg_in[:, i0:i0 + nL, :])
                        for n_i, (inc_id, bglob, a, bb) in enumerate(lst):
                            nc.tensor.matmul(
                                out=pt[:], lhsT=st[:, n_i, :],
                                rhs=gt_[:, bglob - b0, :],
                                start=False,
                                stop=(n_i == nL - 1),
                                skip_group_check=(layer > 0))
                        tmp = work.tile([P, H], f32)
                        nc.vector.tensor_add(out=tmp[:], in0=pt[:],
                                             in1=bias_t[:, layer, :])
                        tmpb = work.tile([P, H], bf16)
                        nc.scalar.activation(out=tmpb[:], in_=tmp[:],
                                             func=AF.Relu)
                        for fh in range(2):
                            ptr_ = pstr.tile([P, P], bf16)
                            nc.tensor.transpose(
                                out=ptr_[:], in_=tmpb[:, fh * P:(fh + 1) * P],
                                identity=ident[:])
                            nc.vector.tensor_copy(
                                out=h_T[:, fh, t * P:(t + 1) * P], in_=ptr_[:])
                    if g + PF < NG:
                        prep(g + PF)
                    if hooks and g in hooks:
                        hooks[g]()

            def emit_m(layer, chunk):
                # M = h @ W for one tile chunk -> MB, then AllGather chunk
                # (strided output: each core's rows land inside its slab)
                t0, t1 = (0, TC0) if chunk == 0 else (TC0, TILES)
                MB = MBs[layer % 2]
                MF = MFs[layer % 2]
                for t in range(t0, t1):
                    pm = psmm.tile([P, H], f32, tag="mm")
                    for fh in range(2):
                        nc.tensor.matmul(
                            out=pm[:],
                            lhsT=h_T[:, fh, t * P:(t + 1) * P],
                            rhs=wl_t[:, layer - 1, fh, :],
                            start=(fh == 0), stop=(fh == 1))
                    mt = work.tile([P, H], bf16)
                    nc.scalar.copy(out=mt[:], in_=pm[:])
                    nc.sync.dma_start(
                        out=MB[t * P:(t + 1) * P, :], in_=mt[:])
                if chunk == 1:
                    nc.gpsimd.collective_compute(
                        "AllGather", mybir.AluOpType.bypass,
                        replica_groups=[list(range(CORES))],
                        ins=[MB[:].opt()],
                        outs=[MF[:, :, :].opt()])

            # ---- pooling state (masked mean/max on h_T, emitted per-graph
            # from spmm(3) hooks as soon as covering tiles are done) ----
            pooled = []
            for fh in range(2):
                mean_t = cpool.tile([P, GPC], f32, tag=f"mean{fh}")
                max_t = cpool.tile([P, GPC], f32, tag=f"max{fh}")
                nc.vector.memset(mean_t[:], 0.0)
                nc.vector.memset(max_t[:], 0.0)
                pooled.append((mean_t, max_t))
            lo_fix, cov_len = meta["lo_fix"], meta["cov_len"]

            def pool_graph(j):
                ln = int(cov_len[j])
                lo = int(lo_fix[j])
                mka = hmp.tile([P, MAXCOV], bf16, tag="mask")
                mkb = hmp.tile([P, MAXCOV], bf16, tag="mask")
                mks = [mka, mkb]
                nc.sync.dma_start(out=mks[0][:, 0:ln],
                                  in_=mask_in[j, :, 0:ln])
                nc.vector.tensor_copy(out=mks[1][:, 0:ln],
                                      in_=mks[0][:, 0:ln])
                for fh in range(2):
                    mk = mks[fh]
                    nc.vector.tensor_mul(out=mk[:, 0:ln],
                                         in0=h_T[:, fh, lo:lo + ln],
                                         in1=mk[:, 0:ln])
                    nc.vector.tensor_reduce(
                        out=pooled[fh][0][:, j:j + 1], in_=mk[:, 0:ln],
                        axis=mybir.AxisListType.X, op=mybir.AluOpType.add)
                    nc.vector.tensor_reduce(
                        out=pooled[fh][1][:, j:j + 1], in_=mk[:, 0:ln],
                        axis=mybir.AxisListType.X, op=mybir.AluOpType.max)

            # graphs ready after each group of spmm(3)
            ready_at = [[] for _ in range(NGRP)]
            for j in range(GPC):
                need = int(lo_fix[j]) + int(cov_len[j])
                g = max(0, min(NGRP - 1, (need + GT * P - 1) // (GT * P) - 1))
                ready_at[g].append(j)

            def pool_hook(g):
                for j in ready_at[g]:
                    pool_graph(j)

            GSPLIT = TC0 // GT - 1   # group after which chunk-0 h is ready
            for layer in (0, 1, 2, 3):
                if layer < 3:
                    hooks = {GSPLIT: (lambda l: lambda: emit_m(l, 0))(layer + 1),
                             NGRP - 1: (lambda l: lambda: emit_m(l, 1))(layer + 1)}
                else:
                    hooks = {g: (lambda gg: lambda: pool_hook(gg))(g)
                             for g in range(NGRP)}
                spmm(layer, hooks=hooks)
            # scale means by 1/cnt, cast to bf16 lhsT chunks
            chunks = []
            for fh in range(2):
                mean_t, max_t = pooled[fh]
                nc.vector.tensor_mul(out=mean_t[:], in0=mean_t[:],
                                     in1=cinv_t[:])
            for (kind, fh) in ((0, 0), (0, 1), (1, 0), (1, 1)):
                src = pooled[fh][kind]
                cb = work.tile([P, GPC], bf16, tag=f"ch{kind}{fh}")
                nc.vector.tensor_copy(out=cb[:], in_=src[:])
                chunks.append(cb)

            # ---- classifier MLP ----

            ph_full = psmm.tile([P, H], f32, tag="mm")
            ph = ph_full[0:GPC, :]
            for k in range(4):
                nc.tensor.matmul(out=ph[:], lhsT=chunks[k][:],
                                 rhs=cw0_t[:, k, :],
                                 start=(k == 0), stop=(k == 3))
            hc1 = work.tile([GPC, H], f32, tag="hc1")
            nc.vector.tensor_add(out=hc1[:], in0=ph[:], in1=cb0_t[:])
            hcb = work.tile([GPC, H], bf16, tag="hcb")
            nc.scalar.activation(out=hcb[:], in_=hc1[:], func=AF.Relu)
            hTt = []
            for k in range(2):
                ptr_ = pstr.tile([P, P], bf16)
                nc.tensor.transpose(out=ptr_[0:P, 0:GPC],
                                    in_=hcb[:, k * P:(k + 1) * P],
                                    identity=ident[0:GPC, 0:GPC])
                ht = work.tile([P, GPC], bf16, tag=f"hTt{k}")
                nc.vector.tensor_copy(out=ht[:], in_=ptr_[0:P, 0:GPC])
                hTt.append(ht)
            pl_full = psmm.tile([P, H], f32, tag="mm")
            pl = pl_full[0:GPC, 0:2]
            for k in range(2):
                nc.tensor.matmul(out=pl[:], lhsT=hTt[k][:],
                                 rhs=cw1_t[:, k, :],
                                 start=(k == 0), stop=(k == 1))
            lg = work.tile([GPC, 2], f32, tag="lg")
            nc.vector.tensor_add(out=lg[:], in0=pl[:], in1=cb1_t[:])
            nc.sync.dma_start(out=out[:, :], in_=lg[:])
    nc.finalize()
    return nc


def kernel(node_ids, edge_index, batch, emb, w0, b0, w1, b1, w2, b2, w3, b3,
           cw0, cb0, cw1, cb1):
    per_core, meta = _preprocess(node_ids, edge_index, batch,
                                 np.asarray(emb, F32))
    nc = _build(meta)

    embT = np.zeros((P, 1024), F32)
    embT[:, :V] = np.asarray(emb, F32).T
    wlk = np.transpose(np.stack([np.asarray(w, F32).reshape(2, P, H)
                    for w in (w1, w2, w3)]), (2, 0, 1, 3)).copy()
    biases = np.stack([np.broadcast_to(np.asarray(b, F32), (P, H))
                       for b in (b0, b1, b2, b3)], axis=1).copy()
    ins = []
    for c in range(CORES):
        pc = per_core[c]
        ins.append(dict(
            idx16=pc["idx16"], seg=pc["seg"], selfn=pc["selfn"],
            xselfT=pc["xselfT"], mask=pc["mask"], cinv=pc["cinv"],
            embT=embT.astype(BF), w0=np.asarray(w0, F32).astype(BF),
            wl=wlk.astype(BF), bias=biases.astype(F32),
            cw0=np.transpose(np.asarray(cw0, F32).reshape(4, P, H), (1, 0, 2)).astype(BF),
            cb0=np.broadcast_to(np.asarray(cb0, F32), (GPC, H)).copy(),
            cw1=np.transpose(np.asarray(cw1, F32).reshape(2, P, 2), (1, 0, 2)).astype(BF),
            cb1=np.broadcast_to(np.asarray(cb1, F32), (GPC, 2)).copy(),
        ))
    trace = False
    try:  # register NTFF hook so exec_time_ns is measurable (best effort)
        import sys, types
        import antenv
        if "antenv.axon_hooks" not in sys.modules:
            hooks = types.ModuleType("antenv.axon_hooks")
            hooks._h = None
            hooks.set_axon_ntff_profile_hook = lambda h: setattr(hooks, "_h", h)
            hooks.get_axon_ntff_profile_hook = lambda: hooks._h
            sys.modules["antenv.axon_hooks"] = hooks
            antenv.axon_hooks = hooks
            from trn_agent_boot.trn_boot import _ntff_profile_via_ctypes
            hk = _ntff_profile_via_ctypes("/opt/axon/libaxon_pjrt.so")
            if hk is not None:
                hooks.set_axon_ntff_profile_hook(hk)
                trace = True
        else:
            trace = True
    except Exception:
        trace = False
    res = run_bass_kernel_spmd(nc, ins, core_ids=list(range(CORES)),
                               trace=trace)
    logits = np.concatenate([res.results[c]["out"] for c in range(CORES)], 0)
    globals()["last_exec_ns"] = res.exec_time_ns
    globals()["last_res"] = res
    return logits.astype(np.float32)



# revision 79
# speedup vs baseline: 2.1860x; 1.0249x over previous
"""GCN (EnhancedTaintFlowGNN) on 8 Trainium2 NeuronCores.

Sharding: 32 graphs/core (batch sorted -> contiguous node ranges). Per GCN
layer: M = h @ W locally (PE), AllGather M, pull-side SpMM via dma_gather
from 8 owner-aligned int16 tables + PE segment-matmuls (norm folded into the
segment coefficients). Self-loops preloaded into PSUM by ACT scaled by
dinv^2. Layer 0 gathers rows of T0 = emb @ w0 (vocab table, replicated on
every core) so no exchange is needed. Pooling: masked DVE mean/max on
transposed features; classifier MLP on PE.
"""
import numpy as np
import ml_dtypes

import concourse.bass as bass
import concourse.bacc as bacc
import concourse.tile as tile
from concourse import mybir
from concourse.bass_utils import run_bass_kernel_spmd
from concourse.masks import make_identity

P = 128
N, E, G, V, D, H = 150000, 300000, 256, 1000, 128, 256
CORES = 8
GPC = G // CORES
GT = 8         # dst-tiles per psum group
MAXCALL = 8    # max 128-slot blocks per dma_gather call
BF = ml_dtypes.bfloat16
F32 = np.float32
PREPARE = False  # prepare_only+trigger races (NaN) — keep direct gathers


def _idx16_layout(idx):
    n = idx.shape[0]
    s = n // 16
    out = np.zeros((P, s), np.int16)
    blk = idx.reshape(s, 16).T.astype(np.int16)
    for g in range(8):
        out[g * 16:(g + 1) * 16, :] = blk
    return out


def _build_stream(counts2d, ngrp, tiles):
    """counts2d [tiles, n_own]. Returns (segs, total_slots).
    segs: per (group, owner): (owner, slot_off, nblk, incidences)
    incidences: (local_blk, tile, a, b) row-ranges inside each block."""
    n_own = counts2d.shape[1]
    segs = []
    off = 0
    for g in range(ngrp):
        tl = list(range(g * GT, min((g + 1) * GT, tiles)))
        for o in range(n_own):
            cnts = [int(counts2d[t, o]) for t in tl]
            seglen = sum(cnts)
            nblk = max(1, (seglen + P - 1) // P)
            inc = []
            r = 0
            for t, k in zip(tl, cnts):
                if k == 0:
                    continue
                r0, r1 = r, r + k
                for b in range(r0 // P, (r1 - 1) // P + 1):
                    a = max(r0 - b * P, 0)
                    bb = min(r1 - b * P, P)
                    if bb > a:
                        inc.append((b, t, a, bb))
                r = r1
            segs.append((o, off, nblk, inc))
            off += nblk * P
    return segs, off


def _make_plan(segs, ngrp, inc_base):
    """Per group: calls [(owner, slot_off, nblk, blk0)] and tiles
    [(t, [(inc_id, bglob)])]; inc ids sequential in builder order."""
    per_g = len(segs) // ngrp
    plan = []
    inc_id = inc_base
    for g in range(ngrp):
        gsegs = segs[g * per_g:(g + 1) * per_g]
        calls = []
        for (o, off, nblk, inc) in gsegs:
            done = 0
            while done < nblk:
                nb = min(MAXCALL, nblk - done)
                calls.append((o, off + done * P, nb, off // P + done))
                done += nb
        tset = sorted({t for (_, _, _, inc) in gsegs for (_, t, _, _) in inc})
        tiles = []
        for t in tset:
            lst = []
            for (o, off, nblk, inc) in gsegs:
                for (b, tt, a, bb) in inc:
                    if tt == t:
                        lst.append((inc_id, off // P + b, a, bb))
                        inc_id += 1
            tiles.append((t, lst))
        plan.append((calls, tiles))
    return plan, inc_id


def _preprocess(node_ids, edge_index, batch, emb_np):
    node_ids = np.asarray(node_ids).astype(np.int64)
    src_g = np.asarray(edge_index)[0].astype(np.int64)
    dst_g = np.asarray(edge_index)[1].astype(np.int64)
    batch = np.asarray(batch).astype(np.int64)

    deg = 1.0 + np.bincount(dst_g, minlength=N).astype(np.float64)
    dinv = (1.0 / np.sqrt(deg)).astype(F32)
    selfnorm = (dinv * dinv).astype(F32)

    gcnt = np.bincount(batch, minlength=G)
    goff = np.zeros(G + 1, np.int64)
    goff[1:] = np.cumsum(gcnt)
    node_start = np.array([int(goff[c * GPC]) for c in range(CORES + 1)])
    n_c = node_start[1:] - node_start[:-1]
    TILES = int(np.ceil(n_c.max() / P))
    SLAB = TILES * P
    assert SLAB <= 32767, SLAB
    NGRP = (TILES + GT - 1) // GT

    e_oc = np.searchsorted(node_start[1:], dst_g, side="right")
    e_os = np.searchsorted(node_start[1:], src_g, side="right")

    # chunked AllGather split point (collective only; streams stay per-owner)
    TC0 = (TILES // 2 + GT - 1) // GT * GT
    R0, R1 = TC0 * P, SLAB - TC0 * P

    cores = []
    for c in range(CORES):
        m = e_oc == c
        cores.append(dict(src=src_g[m], dstl=dst_g[m] - node_start[c],
                          own=e_os[m], n=int(n_c[c]), nlo=int(node_start[c])))

    # uniform per-(tile, owner) counts
    K1 = np.zeros((TILES, CORES), np.int64)
    K0 = np.zeros((TILES, 1), np.int64)
    for c in range(CORES):
        t_of = cores[c]["dstl"] // P
        cnt = np.zeros((TILES, CORES), np.int64)
        np.add.at(cnt, (t_of, cores[c]["own"]), 1)
        np.maximum(K1, cnt, out=K1)
        c0 = np.bincount(t_of, minlength=TILES).reshape(-1, 1)
        np.maximum(K0, c0, out=K0)

    segs1, S1 = _build_stream(K1, NGRP, TILES)
    segs0, S0 = _build_stream(K0, NGRP, TILES)
    NB0, NB1 = S0 // P, S1 // P
    plan0, ninc0 = _make_plan(segs0, NGRP, 0)
    plan1, ninc1 = _make_plan(segs1, NGRP, ninc0)

    # graph cover ranges (uniform across cores)
    glo_all = np.stack([goff[c * GPC:(c + 1) * GPC] - node_start[c]
                        for c in range(CORES)])   # [CORES, GPC]
    ghi_all = np.stack([goff[c * GPC + 1:(c + 1) * GPC + 1] - node_start[c]
                        for c in range(CORES)])
    lo_fix = glo_all.min(0)
    hi_fix = ghi_all.max(0)
    MAXCOV = int((hi_fix - lo_fix).max())
    MAXCOV = (MAXCOV + 31) // 32 * 32
    cov_len = np.minimum(MAXCOV, SLAB - lo_fix)

    per_core = []
    for c in range(CORES):
        cd = cores[c]
        t_of = cd["dstl"] // P
        idx0 = np.zeros(S0, np.int16)
        idx1 = np.zeros(S1, np.int16)
        scol0 = np.zeros(S0, np.int64)
        scoef0 = np.zeros(S0, F32)
        scol1 = np.zeros(S1, np.int64)
        scoef1 = np.zeros(S1, F32)

        # --- L123 stream fill ---
        order = np.lexsort((cd["dstl"], t_of, cd["own"], t_of // GT))
        srcs = cd["src"][order]
        dstl = cd["dstl"][order]
        owns = cd["own"][order]
        tof = t_of[order]
        coefs = dinv[srcs] * dinv[dstl + cd["nlo"]]
        ptr = 0
        for (o, off, nblk, inc) in segs1:
            starts = {}
            for (b, t, a, bb) in inc:
                if t not in starts:
                    starts[t] = off + b * P + a
            for t in sorted(starts):
                p = starts[t]
                while ptr < len(srcs) and tof[ptr] == t and owns[ptr] == o:
                    idx1[p] = srcs[ptr] - node_start[o]
                    scol1[p] = dstl[ptr] % P
                    scoef1[p] = coefs[ptr]
                    p += 1
                    ptr += 1
        assert ptr == len(srcs), (c, ptr, len(srcs))

        # --- L0 stream fill (edges only; table = T0 by vocab id) ---
        order0 = np.lexsort((cd["dstl"],))
        s0 = cd["src"][order0]
        d0 = cd["dstl"][order0]
        c0 = dinv[s0] * dinv[d0 + cd["nlo"]]
        ptr = 0
        for (o, off, nblk, inc) in segs0:
            starts = {}
            for (b, t, a, bb) in inc:
                if t not in starts:
                    starts[t] = off + b * P + a
            for t in sorted(starts):
                p = starts[t]
                while ptr < len(s0) and d0[ptr] // P == t:
                    idx0[p] = node_ids[s0[ptr]]
                    scol0[p] = d0[ptr] % P
                    scoef0[p] = c0[ptr]
                    p += 1
                    ptr += 1
        assert ptr == len(s0)

        # self-loop term via host-precomputed scaled embedding rows
        xs = np.zeros((SLAB, D), F32)
        ids_own = node_ids[cd["nlo"]:cd["nlo"] + cd["n"]]
        xs[:cd["n"], :] = emb_np[ids_own, :] * \
            selfnorm[cd["nlo"]:cd["nlo"] + cd["n"], None]
        xselfT = np.ascontiguousarray(xs.T.astype(BF))

        seg = np.zeros((ninc1, P, P), F32)
        for plan, scol, scoef in ((plan0, scol0, scoef0),
                                  (plan1, scol1, scoef1)):
            for (calls, tiles) in plan:
                for (t, lst) in tiles:
                    for (inc_id, bglob, a, bb) in lst:
                        rows = np.arange(a, bb)
                        slots = bglob * P + rows
                        seg[inc_id, rows, scol[slots]] = scoef[slots]

        selfn = np.zeros((P, TILES), F32)
        ar = np.arange(cd["n"])
        selfn[ar % P, ar // P] = selfnorm[cd["nlo"]:cd["nlo"] + cd["n"]]

        # graph masks [GPC, P, MAXCOV] bf16 (replicated across partitions)
        mask = np.zeros((GPC, MAXCOV), F32)
        cinv = np.zeros(GPC, F32)
        for j in range(GPC):
            lo = int(glo_all[c, j]) - int(lo_fix[j])
            hi = int(ghi_all[c, j]) - int(lo_fix[j])
            mask[j, lo:hi] = 1.0
            cnt = int(ghi_all[c, j] - glo_all[c, j])
            cinv[j] = 1.0 / max(cnt, 1)
        maskb = np.broadcast_to(mask[:, None, :], (GPC, P, MAXCOV))

        per_core.append(dict(
            idx16=_idx16_layout(np.concatenate([idx0, idx1]).astype(np.int16)),
            seg=np.ascontiguousarray(seg.astype(BF).transpose(1, 0, 2)),
            xselfT=xselfT,
            selfn=selfn,
            mask=np.ascontiguousarray(maskb.astype(BF)),
            cinv=np.broadcast_to(cinv, (P, GPC)).copy(),
        ))

    meta = dict(TILES=TILES, SLAB=SLAB, NGRP=NGRP, NB0=NB0, NB1=NB1,
                S0=S0, S1=S1, segs0=segs0, segs1=segs1,
                plan0=plan0, plan1=plan1, NINC=ninc1,
                lo_fix=lo_fix, cov_len=cov_len, MAXCOV=MAXCOV,
                TC0=TC0, R0=R0, R1=R1)
    return per_core, meta


def _build(meta):
    TILES, SLAB, NGRP = meta["TILES"], meta["SLAB"], meta["NGRP"]
    NB0, NB1 = meta["NB0"], meta["NB1"]
    MAXCOV = meta["MAXCOV"]
    f32, bf16, i16 = mybir.dt.float32, mybir.dt.bfloat16, mybir.dt.int16
    AF = mybir.ActivationFunctionType
    nc = bacc.Bacc("TRN2", target_bir_lowering=False, debug=False,
                   num_devices=CORES)

    SCOLS = (meta["S0"] + meta["S1"]) // 16
    idx_in = nc.dram_tensor("idx16", [P, SCOLS], i16, kind="ExternalInput")
    seg_in = nc.dram_tensor("seg", [P, meta["NINC"], P], bf16,
                            kind="ExternalInput")
    selfn_in = nc.dram_tensor("selfn", [P, TILES], f32, kind="ExternalInput")
    xselfT_in = nc.dram_tensor("xselfT", [P, SLAB], bf16,
                               kind="ExternalInput")
    mask_in = nc.dram_tensor("mask", [GPC, P, MAXCOV], bf16,
                             kind="ExternalInput")
    cinv_in = nc.dram_tensor("cinv", [P, GPC], f32, kind="ExternalInput")
    embT_in = nc.dram_tensor("embT", [P, 1024], bf16, kind="ExternalInput")
    w0_in = nc.dram_tensor("w0", [P, H], bf16, kind="ExternalInput")
    wl_in = nc.dram_tensor("wl", [P, 3, 2, H], bf16, kind="ExternalInput")
    b_in = nc.dram_tensor("bias", [P, 4, H], f32, kind="ExternalInput")
    cw0_in = nc.dram_tensor("cw0", [P, 4, H], bf16, kind="ExternalInput")
    cb0_in = nc.dram_tensor("cb0", [GPC, H], f32, kind="ExternalInput")
    cw1_in = nc.dram_tensor("cw1", [P, 2, 2], bf16, kind="ExternalInput")
    cb1_in = nc.dram_tensor("cb1", [GPC, 2], f32, kind="ExternalInput")
    out = nc.dram_tensor("out", [GPC, 2], f32, kind="ExternalOutput")

    T0_d = nc.dram_tensor("T0d", [1024, H], bf16)

    with tile.TileContext(nc) as tc:
        with (
            tc.tile_pool(name="const", bufs=1) as cpool,
            tc.tile_pool(name="gat", bufs=4) as gpool,
            tc.tile_pool(name="segp", bufs=3) as segp,
            tc.tile_pool(name="selfp", bufs=4) as selfp,
            tc.tile_pool(name="work", bufs=4) as work,
            tc.tile_pool(name="hmp", bufs=2) as hmp,
            tc.tile_pool(name="ps", bufs=4, space="PSUM") as ps,
            tc.tile_pool(name="pstr", bufs=2, space="PSUM") as pstr,
            tc.tile_pool(name="psmm", bufs=2, space="PSUM") as psmm,
            tc.tile_pool(name="dram", bufs=1, space="DRAM") as dpool,
        ):
            idx_t = cpool.tile([P, SCOLS], i16)
            nc.sync.dma_start(out=idx_t[:], in_=idx_in[:, :])
            selfn_t = cpool.tile([P, TILES], f32)
            nc.sync.dma_start(out=selfn_t[:], in_=selfn_in[:, :])
            cinv_t = cpool.tile([P, GPC], f32)
            nc.sync.dma_start(out=cinv_t[:], in_=cinv_in[:, :])
            embT_t = cpool.tile([P, 1024], bf16)
            nc.sync.dma_start(out=embT_t[:], in_=embT_in[:, :])
            w0_t = cpool.tile([P, H], bf16)
            nc.sync.dma_start(out=w0_t[:], in_=w0_in[:, :])
            wl_t = cpool.tile([P, 3, 2, H], bf16)
            nc.sync.dma_start(out=wl_t[:], in_=wl_in[:, :, :, :])
            bias_t = cpool.tile([P, 4, H], f32)
            nc.sync.dma_start(out=bias_t[:], in_=b_in[:, :, :])
            cw0_t = cpool.tile([P, 4, H], bf16)
            nc.sync.dma_start(out=cw0_t[:], in_=cw0_in[:, :, :])
            cb0_t = cpool.tile([GPC, H], f32)
            nc.sync.dma_start(out=cb0_t[:], in_=cb0_in[:, :])
            cw1_t = cpool.tile([P, 2, 2], bf16)
            nc.sync.dma_start(out=cw1_t[:], in_=cw1_in[:, :, :])
            cb1_t = cpool.tile([GPC, 2], f32)
            nc.sync.dma_start(out=cb1_t[:], in_=cb1_in[:, :])

            h_T = nc.alloc_sbuf_tensor("hT", [P, 2, SLAB], bf16)
            ident = cpool.tile([P, P], bf16)
            make_identity(nc, ident[:])

            # ---- T0 = emb @ w0 ----
            for vb in range(8):
                pt0 = psmm.tile([P, H], f32, tag="mm")
                nc.tensor.matmul(out=pt0[:], lhsT=embT_t[:, vb * P:(vb + 1) * P],
                                 rhs=w0_t[:], start=True, stop=True)
                t0s = work.tile([P, H], bf16)
                nc.scalar.copy(out=t0s[:], in_=pt0[:])
                nc.sync.dma_start(out=T0_d[vb * P:(vb + 1) * P, :], in_=t0s[:])

            TC0, R0, R1 = meta["TC0"], meta["R0"], meta["R1"]
            # [parity] double-buffered exchange tensors: the AllGathers for
            # layer L+1 fire while spmm(L) still reads layer L's data
            MBs, MFs = [], []
            for par in range(2):
                mb = nc.dram_tensor(f"MBd{par}", [SLAB, H], bf16)
                mf = nc.dram_tensor(f"MFd{par}", [CORES, SLAB, H], bf16,
                                    addr_space="Shared")
                MBs.append(mb)
                MFs.append(mf)

            # one semaphore per SWDGE lane, cycled in Pool-DMA emission order
            # to match tile_sem_assignment's next_sw_dma_idx rotation
            dma_sems = [nc.alloc_semaphore(f"swdge_dma{i}") for i in range(8)]
            prep_ct = [0]

            def gmeta(plan):
                out = []
                for (calls, tiles) in plan:
                    b0 = min(c[3] for c in calls)
                    nb = sum(c[2] for c in calls)
                    out.append((b0, nb))
                return out
            gm0, gm1 = gmeta(meta["plan0"]), gmeta(meta["plan1"])
            NBGMAX = max(nb for (_, nb) in gm0 + gm1)
            PF = 4  # gather group prefetch depth == gpool bufs

            def spmm(layer, hooks=None):
                plan = meta["plan0"] if layer == 0 else meta["plan1"]
                gmv = gm0 if layer == 0 else gm1
                base_slot = 0 if layer == 0 else meta["S0"]
                NG = len(plan)
                gt_map = {}

                def prep(g):
                    calls, _ = plan[g]
                    b0, _ = gmv[g]
                    gt_ = gpool.tile([P, NBGMAX, H], bf16, tag="gtg")
                    gt_map[g] = (gt_, b0)
                    for (o2, soff, nb, blk0) in calls:
                        slot0 = base_slot + soff
                        if layer == 0:
                            tab = T0_d[0:1024, :]
                        else:
                            tab = MFs[layer % 2][o2, :, :]
                        sem = dma_sems[prep_ct[0] % 8] if PREPARE else None
                        prep_ct[0] += 1
                        nc.gpsimd.dma_gather(
                            out_ap=gt_[:, blk0 - b0:blk0 - b0 + nb, :],
                            in_ap=tab,
                            idxs_ap=idx_t[:, slot0 // 16:
                                          (slot0 + nb * P) // 16],
                            num_idxs=nb * P, num_idxs_reg=nb * P,
                            elem_size=H, queue_num=0,
                            single_packet=False,
                            prepare_only=PREPARE, sem=sem)
                    if PREPARE:
                        nc.gpsimd.trigger_dma(count=None)

                for g in range(min(PF, NG)):
                    prep(g)
                for g, (calls, tiles) in enumerate(plan):
                    gt_, b0 = gt_map.pop(g)
                    tmap = dict(tiles)
                    for t in range(g * GT, min((g + 1) * GT, TILES)):
                        lst = tmap.get(t, [])
                        nL = len(lst)
                        pt = ps.tile([P, H], f32)
                        if layer > 0:
                            ssrc = selfp.tile([P, H], bf16, tag="ss")
                            nc.sync.dma_start(
                                out=ssrc[:],
                                in_=MBs[layer % 2][t * P:(t + 1) * P, :])
                            nc.scalar.activation(
                                out=pt[:], in_=ssrc[:], func=AF.Copy,
                                scale=selfn_t[:, t:t + 1])
                        else:
                            xs = selfp.tile([P, P], bf16, tag="xs")
                            nc.sync.dma_start(
                                out=xs[:],
                                in_=xselfT_in[:, t * P:(t + 1) * P])
                            nc.tensor.matmul(
                                out=pt[:], lhsT=xs[:], rhs=w0_t[:],
                                start=True, stop=(nL == 0))
                        if nL:
                            i0 = lst[0][0]
                            st = segp.tile([P, nL, P], bf16, tag="st")
                            nc.scalar.dma_start(out=st[:, :, :],
                                                in_=seg_in[:, i0:i0 + nL, :])
                        for n_i, (inc_id, bglob, a, bb) in enumerate(lst):
                            nc.tensor.matmul(
                                out=pt[:], lhsT=st[:, n_i, :],
                                rhs=gt_[:, bglob - b0, :],
                                start=False,
                                stop=(n_i == nL - 1),
                                skip_group_check=(layer > 0))
                        tmp = work.tile([P, H], f32)
                        nc.vector.tensor_add(out=tmp[:], in0=pt[:],
                                             in1=bias_t[:, layer, :])
                        tmpb = work.tile([P, H], bf16)
                        nc.scalar.activation(out=tmpb[:], in_=tmp[:],
                                             func=AF.Relu)
                        for fh in range(2):
                            ptr_ = pstr.tile([P, P], bf16)
                            nc.tensor.transpose(
                                out=ptr_[:], in_=tmpb[:, fh * P:(fh + 1) * P],
                                identity=ident[:])
                            nc.vector.tensor_copy(
                                out=h_T[:, fh, t * P:(t + 1) * P], in_=ptr_[:])
                    if g + PF < NG:
                        prep(g + PF)
                    if hooks and g in hooks:
                        hooks[g]()

            def emit_m(layer, chunk):
                # M = h @ W for one tile chunk -> MB, then AllGather chunk
                # (strided output: each core's rows land inside its slab)
                t0, t1 = (0, TC0) if chunk == 0 else (TC0, TILES)
                MB = MBs[layer % 2]
                MF = MFs[layer % 2]
                for t in range(t0, t1):
                    pm = psmm.tile([P, H], f32, tag="mm")
                    for fh in range(2):
                        nc.tensor.matmul(
                            out=pm[:],
                            lhsT=h_T[:, fh, t * P:(t + 1) * P],
                            rhs=wl_t[:, layer - 1, fh, :],
                            start=(fh == 0), stop=(fh == 1))
                    mt = work.tile([P, H], bf16)
                    nc.scalar.copy(out=mt[:], in_=pm[:])
                    nc.sync.dma_start(
                        out=MB[t * P:(t + 1) * P, :], in_=mt[:])
                if chunk == 1:
                    nc.gpsimd.collective_compute(
                        "AllGather", mybir.AluOpType.bypass,
                        replica_groups=[list(range(CORES))],
                        ins=[MB[:].opt()],
                        outs=[MF[:, :, :].opt()])

            # ---- pooling state (masked mean/max on h_T, emitted per-graph
            # from spmm(3) hooks as soon as covering tiles are done) ----
            pooled = []
            for fh in range(2):
                mean_t = cpool.tile([P, GPC], f32, tag=f"mean{fh}")
                max_t = cpool.tile([P, GPC], f32, tag=f"max{fh}")
                nc.vector.memset(mean_t[:], 0.0)
                nc.vector.memset(max_t[:], 0.0)
                pooled.append((mean_t, max_t))
            lo_fix, cov_len = meta["lo_fix"], meta["cov_len"]

            def pool_graph(j):
                ln = int(cov_len[j])
                lo = int(lo_fix[j])
                mka = hmp.tile([P, MAXCOV], bf16, tag="mask")
                mkb = hmp.tile([P, MAXCOV], bf16, tag="mask")
                mks = [mka, mkb]
                nc.sync.dma_start(out=mks[0][:, 0:ln],
                                  in_=mask_in[j, :, 0:ln])
                nc.vector.tensor_copy(out=mks[1][:, 0:ln],
                                      in_=mks[0][:, 0:ln])
                for fh in range(2):
                    mk = mks[fh]
                    nc.vector.tensor_mul(out=mk[:, 0:ln],
                                         in0=h_T[:, fh, lo:lo + ln],
                                         in1=mk[:, 0:ln])
                    nc.vector.tensor_reduce(
                        out=pooled[fh][0][:, j:j + 1], in_=mk[:, 0:ln],
                        axis=mybir.AxisListType.X, op=mybir.AluOpType.add)
                    nc.vector.tensor_reduce(
                        out=pooled[fh][1][:, j:j + 1], in_=mk[:, 0:ln],
                        axis=mybir.AxisListType.X, op=mybir.AluOpType.max)

            # graphs ready after each group of spmm(3)
            ready_at = [[] for _ in range(NGRP)]
            for j in range(GPC):
                need = int(lo_fix[j]) + int(cov_len[j])
                g = max(0, min(NGRP - 1, (need + GT * P - 1) // (GT * P) - 1))
                ready_at[g].append(j)

            def pool_hook(g):
                for j in ready_at[g]:
                    pool_graph(j)

            GSPLIT = TC0 // GT - 1   # group after which chunk-0 h is ready
            for layer in (0, 1, 2, 3):
                if layer < 3:
                    hooks = {GSPLIT: (lambda l: lambda: emit_m(l, 0))(layer + 1),
                             NGRP - 1: (lambda l: lambda: emit_m(l, 1))(layer + 1)}
                else:
                    hooks = {g: (lambda gg: lambda: pool_hook(gg))(g)
                             for g in range(NGRP)}
                spmm(layer, hooks=hooks)
            # scale means by 1/cnt, cast to bf16 lhsT chunks
            chunks = []
            for fh in range(2):
                mean_t, max_t = pooled[fh]
                nc.vector.tensor_mul(out=mean_t[:], in0=mean_t[:],
                                     in1=cinv_t[:])
            for (kind, fh) in ((0, 0), (0, 1), (1, 0), (1, 1)):
                src = pooled[fh][kind]
                cb = work.tile([P, GPC], bf16, tag=f"ch{kind}{fh}")
                nc.vector.tensor_copy(out=cb[:], in_=src[:])
                chunks.append(cb)

            # ---- classifier MLP ----

            ph_full = psmm.tile([P, H], f32, tag="mm")
            ph = ph_full[0:GPC, :]
            for k in range(4):
                nc.tensor.matmul(out=ph[:], lhsT=chunks[k][:],
                                 rhs=cw0_t[:, k, :],
                                 start=(k == 0), stop=(k == 3))
            hc1 = work.tile([GPC, H], f32, tag="hc1")
            nc.vector.tensor_add(out=hc1[:], in0=ph[:], in1=cb0_t[:])
            hcb = work.tile([GPC, H], bf16, tag="hcb")
            nc.scalar.activation(out=hcb[:], in_=hc1[:], func=AF.Relu)
            hTt = []
            for k in range(2):
                ptr_ = pstr.tile([P, P], bf16)
                nc.tensor.transpose(out=ptr_[0:P, 0:GPC],
                                    in_=hcb[:, k * P:(k + 1) * P],
                                    identity=ident[0:GPC, 0:GPC])
                ht = work.tile([P, GPC], bf16, tag=f"hTt{k}")
                nc.vector.tensor_copy(out=ht[:], in_=ptr_[0:P, 0:GPC])
                hTt.append(ht)
            pl_full = psmm.tile([P, H], f32, tag="mm")
            pl = pl_full[0:GPC, 0:2]
            for k in range(2):
                nc.tensor.matmul(out=pl[:], lhsT=hTt[k][:],
                                 rhs=cw1_t[:, k, :],
                                 start=(k == 0), stop=(k == 1))
            lg = work.tile([GPC, 2], f32, tag="lg")
            nc.vector.tensor_add(out=lg[:], in0=pl[:], in1=cb1_t[:])
            nc.sync.dma_start(out=out[:, :], in_=lg[:])
    nc.finalize()
    return nc


def kernel(node_ids, edge_index, batch, emb, w0, b0, w1, b1, w2, b2, w3, b3,
           cw0, cb0, cw1, cb1):
    per_core, meta = _preprocess(node_ids, edge_index, batch,
                                 np.asarray(emb, F32))
    nc = _build(meta)

    embT = np.zeros((P, 1024), F32)
    embT[:, :V] = np.asarray(emb, F32).T
    wlk = np.transpose(np.stack([np.asarray(w, F32).reshape(2, P, H)
                    for w in (w1, w2, w3)]), (2, 0, 1, 3)).copy()
    biases = np.stack([np.broadcast_to(np.asarray(b, F32), (P, H))
                       for b in (b0, b1, b2, b3)], axis=1).copy()
    ins = []
    for c in range(CORES):
        pc = per_core[c]
        ins.append(dict(
            idx16=pc["idx16"], seg=pc["seg"], selfn=pc["selfn"],
            xselfT=pc["xselfT"], mask=pc["mask"], cinv=pc["cinv"],
            embT=embT.astype(BF), w0=np.asarray(w0, F32).astype(BF),
            wl=wlk.astype(BF), bias=biases.astype(F32),
            cw0=np.transpose(np.asarray(cw0, F32).reshape(4, P, H), (1, 0, 2)).astype(BF),
            cb0=np.broadcast_to(np.asarray(cb0, F32), (GPC, H)).copy(),
            cw1=np.transpose(np.asarray(cw1, F32).reshape(2, P, 2), (1, 0, 2)).astype(BF),
            cb1=np.broadcast_to(np.asarray(cb1, F32), (GPC, 2)).copy(),
        ))
    trace = False
    try:  # register NTFF hook so exec_time_ns is measurable (best effort)
        import sys, types
        import antenv
        if "antenv.axon_hooks" not in sys.modules:
            hooks = types.ModuleType("antenv.axon_hooks")
            hooks._h = None
            hooks.set_axon_ntff_profile_hook = lambda h: setattr(hooks, "_h", h)
            hooks.get_axon_ntff_profile_hook = lambda: hooks._h
            sys.modules["antenv.axon_hooks"] = hooks
            antenv.axon_hooks = hooks
            from trn_agent_boot.trn_boot import _ntff_profile_via_ctypes
            hk = _ntff_profile_via_ctypes("/opt/axon/libaxon_pjrt.so")
            if hk is not None:
                hooks.set_axon_ntff_profile_hook(hk)
                trace = True
        else:
            trace = True
    except Exception:
        trace = False
    res = run_bass_kernel_spmd(nc, ins, core_ids=list(range(CORES)),
                               trace=trace)
    logits = np.concatenate([res.results[c]["out"] for c in range(CORES)], 0)
    globals()["last_exec_ns"] = res.exec_time_ns
    globals()["last_res"] = res
    return logits.astype(np.float32)

